# revision 1
# baseline (speedup 1.0000x reference)
"""ChebyNet (K=1) dual-branch MLP + BN kernel for 8 Trainium2 NeuronCores.

Network (per reference):
  branch b in {1,2}:  h = relu(BN(x_b @ W1_b)) ; h = relu(BN(h @ W2_b)) ; f_b = h @ Wf_b + bf_b
  out = relu(concat(f_1, f_2) @ Wh1 + bh1) @ Wh2 + bh2

ChebConv with K=1 ignores edge_index/edge_weight entirely.  Training-mode
BatchNorm over the node axis makes the linear-layer biases b1/b2 cancel
exactly, so they are never loaded.

Sharding: nodes (axis 0) split across 8 cores, 12500 each, zero-padded to
12544 = 98*128.  Weights replicated.  BN batch stats are combined with an
AllReduce(add) of per-core (sum, sumsq) over the 8 cores; one collective
per (layer, branch), interleaved so each hides under the next branch's
compute.

On-chip layout is feature-major ("transposed"): activations live as
[feat_partition, node_free]; BN reduces along the free axis; weights are
the stationary matmul operand.  Matmul dtypes: layer 1 in bf16 (x is also
kept resident as bf16 xT), everything else float32r (tf32-like).

Layer-1 BN stats use the Gram identity: sumsq(pre1) = diag(W1^T (X^T X) W1)
and sum(pre1) = W1^T (X^T 1), so pass 1 never computes pre1 at all —
X^T X accumulates on the PE from the natural-layout x tiles while they are
transposed for the resident xT.
"""

import os

os.environ.setdefault("JAX_PLATFORMS", "axon,cpu")

import numpy as np

import concourse.bacc as bacc
import concourse.mybir as mybir
import concourse.tile as tile
import concourse.masks as masks
from concourse import bass_utils
from concourse.bass import ts

F32 = mybir.dt.float32
F32R = mybir.dt.float32r
BF16 = mybir.dt.bfloat16
AF = mybir.ActivationFunctionType

NTOT = 100000          # true node count
NCORES = 8
NSH = NTOT // NCORES   # 12500 true nodes per core
NP = 12544             # padded per-core nodes (= 98 * 128)
T = 512                # node-chunk size (free dim of matmuls / PSUM bank)
CHUNKS = [(i * T, T) for i in range(NP // T)] + ([(NP - NP % T, NP % T)] if NP % T else [])
C = len(CHUNKS)
SUPER = 1024           # pass-1 x-load granularity
SCH = [(i * SUPER, SUPER) for i in range(NP // SUPER)]
if NP % SUPER:
    SCH.append((NP - NP % SUPER, NP % SUPER))
PAD0 = NSH - (NP - (NP % T or T))  # first padded column inside last chunk (212)
EPS = 1e-5

_CACHE = {}


def _build_program():
    nc = bacc.Bacc("TRN2", target_bir_lowering=False, debug=False,
                   num_devices=NCORES)

    # ---- kernel I/O -----------------------------------------------------
    xd = [nc.dram_tensor(f"x_{b+1}", [NP, 128], F32R, kind="ExternalInput")
          for b in range(2)]
    w1d = [nc.dram_tensor(f"W1_{b+1}", [128, 512], F32, kind="ExternalInput")
           for b in range(2)]
    w2d = [nc.dram_tensor(f"W2_{b+1}", [512, 512], F32R, kind="ExternalInput")
           for b in range(2)]
    wfd = [nc.dram_tensor(f"Wf_{b+1}", [512, 512], F32R, kind="ExternalInput")
           for b in range(2)]
    bfd = [nc.dram_tensor(f"bf_{b+1}", [512], F32, kind="ExternalInput")
           for b in range(2)]
    gd = [[nc.dram_tensor(f"g{l+1}_{b+1}", [512], F32, kind="ExternalInput")
           for b in range(2)] for l in range(2)]
    bed = [[nc.dram_tensor(f"be{l+1}_{b+1}", [512], F32, kind="ExternalInput")
            for b in range(2)] for l in range(2)]
    wh1d = nc.dram_tensor("Wh1", [1024, 512], F32R, kind="ExternalInput")
    bh1d = nc.dram_tensor("bh1", [512], F32, kind="ExternalInput")
    wh2d = nc.dram_tensor("Wh2", [512, 10], F32R, kind="ExternalInput")
    bh2d = nc.dram_tensor("bh2", [10], F32, kind="ExternalInput")
    auxd = nc.dram_tensor("AUX", [128, 132], F32R, kind="ExternalInput")
    epsd = nc.dram_tensor("EPSA", [128, 1], F32, kind="ExternalInput")
    outd = nc.dram_tensor("OUT", [10, NP], F32, kind="ExternalOutput")

    # ---- DRAM scratch ---------------------------------------------------
    spill = nc.dram_tensor("pre2_spill", [128, 2, 4, NP], BF16)
    cc_in = [[nc.dram_tensor(f"cc{l}{b}_in", [128, 4, 2], F32) for b in range(2)]
             for l in range(2)]
    cc_out = [[nc.dram_tensor(f"cc{l}{b}_out", [128, 4, 2], F32,
                              addr_space="Shared") for b in range(2)]
              for l in range(2)]

    def vec_ap(h, p=128):
        return h.ap().rearrange("(m p) -> p m", p=p)

    with tile.TileContext(nc) as tc:
        with (
            tc.tile_pool(name="wpool", bufs=1) as wp,
            tc.tile_pool(name="stat", bufs=1) as stat,
            tc.tile_pool(name="pf3", bufs=5) as pf3,
        ):
            PF3 = 5
            pf3_tiles = {}
            # ---- constants come in via DMA (on-chip memset/affine_select
            # on gpsimd costs ~50us of startup critical path) -------------
            aux_sb = wp.tile([128, 132], F32R, name="aux_sb")
            nc.sync.dma_start(aux_sb[:], auxd[:, :])
            identr = aux_sb[:, 0:128]
            ones_r = aux_sb[:, 128:132]
            eps_t = stat.tile([128, 1], F32, name="eps_t")
            nc.scalar.dma_start(eps_t[:], epsd[:, :])

            # W1 now (pass-1 projection needs it); everything else deferred
            # into pass 1/2 so startup isn't queued behind weight DMAs.
            w1_bf, w1_r = [], []
            for b in range(2):
                w1f = wp.tile([128, 512], F32, name=f"w1f_{b}")
                nc.scalar.dma_start(w1f[:], w1d[b][:, :])
                w1b = wp.tile([128, 512], BF16, name=f"w1b_{b}")
                nc.vector.tensor_copy(w1b[:], w1f[:])
                w1r = wp.tile([128, 512], F32R, name=f"w1r_{b}")
                nc.vector.tensor_copy(w1r[:], w1b[:])
                w1_bf.append(w1b)
                w1_r.append(w1r)

            # tiles declared up front, DMAs emitted later via the loaders
            w2_t = [wp.tile([128, 4, 512], F32R, name=f"w2_{b}") for b in range(2)]
            wf_t = [wp.tile([128, 4, 512], F32R, name=f"wf_{b}") for b in range(2)]
            bf_sb = [wp.tile([128, 4], F32, name=f"bf_{b}") for b in range(2)]
            wh1_t = wp.tile([128, 8, 512], F32R, name="wh1_t")
            wh2_t = wp.tile([128, 4, 10], F32R, name="wh2_t")
            bh1_sb = wp.tile([128, 4], F32, name="bh1_sb")
            bh2_sb = wp.tile([10, 1], F32, name="bh2_sb")
            g_sb = [stat.tile([128, 2, 4], F32, name=f"g_sb{l}") for l in range(2)]
            be_sb = [stat.tile([128, 2, 4], F32, name=f"be_sb{l}") for l in range(2)]

            def load_pass2_weights():
                for b in range(2):
                    nc.scalar.dma_start(
                        w2_t[b][:], w2d[b].ap().rearrange("(k p) m -> p k m", p=128))
                for b in range(2):
                    nc.scalar.dma_start(g_sb[0][:, b, :], vec_ap(gd[0][b]))
                    nc.scalar.dma_start(be_sb[0][:, b, :], vec_ap(bed[0][b]))

            def load_pass3_weights():
                for b in range(2):
                    nc.scalar.dma_start(
                        wf_t[b][:], wfd[b].ap().rearrange("(k p) m -> p k m", p=128))
                    nc.scalar.dma_start(bf_sb[b][:], vec_ap(bfd[b]))
                    nc.scalar.dma_start(g_sb[1][:, b, :], vec_ap(gd[1][b]))
                    nc.scalar.dma_start(be_sb[1][:, b, :], vec_ap(bed[1][b]))
                nc.scalar.dma_start(
                    wh1_t[:], wh1d.ap().rearrange("(k p) m -> p k m", p=128))
                nc.scalar.dma_start(
                    wh2_t[:], wh2d.ap().rearrange("(k p) m -> p k m", p=128))
                nc.scalar.dma_start(bh1_sb[:], vec_ap(bh1d))
                nc.scalar.dma_start(bh2_sb[:],
                                    bh2d.ap().rearrange("(m o) -> m o", o=1))

            st2 = stat.tile([128, 2, 4, C, 6], F32, name="st2")
            xs_sl = stat.tile([128, 2, len(SCH)], F32, name="xs_sl")
            pay = [[stat.tile([128, 4, 2], F32, name=f"pay{l}{b}")
                    for b in range(2)] for l in range(2)]
            scale_t = [stat.tile([128, 2, 4], F32, name=f"scale{l}") for l in range(2)]
            shift_t = [stat.tile([128, 2, 4], F32, name=f"shift{l}") for l in range(2)]

            def issue_allreduce(l, b):
                nc.sync.dma_start(cc_in[l][b][:, :, :], pay[l][b][:])
                nc.gpsimd.collective_compute(
                    "AllReduce", mybir.AluOpType.add,
                    replica_groups=[list(range(NCORES))],
                    ins=[cc_in[l][b].ap().opt()], outs=[cc_out[l][b].ap().opt()],
                )

            gl_tiles = {}

            def load_stats(l, b, dma_engine=None):
                gl = stat.tile([128, 4, 2], F32, tag=f"gl{l}{b}", name=f"gl{l}{b}")
                (dma_engine or nc.sync).dma_start(gl[:], cc_out[l][b][:, :, :])
                gl_tiles[(l, b)] = gl

            def finish_stats(l, b):
                """cc_out[l][b] -> scale_t[l][:, b, :], shift_t[l][:, b, :]."""
                if (l, b) not in gl_tiles:
                    load_stats(l, b)
                gl = gl_tiles.pop((l, b))
                mu = stat.tile([128, 4], F32, tag="mu", name=f"mu{l}{b}")
                var = stat.tile([128, 4], F32, tag="var", name=f"var{l}{b}")
                tmp = stat.tile([128, 4], F32, tag="tmpf", name=f"tmp{l}{b}")
                nc.vector.tensor_scalar_mul(mu[:], gl[:, :, 0], 1.0 / NTOT)
                nc.vector.tensor_scalar_mul(var[:], gl[:, :, 1], 1.0 / NTOT)
                nc.vector.tensor_mul(tmp[:], mu[:], mu[:])
                nc.vector.tensor_sub(var[:], var[:], tmp[:])
                nc.scalar.activation(var[:], var[:], AF.Sqrt, bias=eps_t[:])
                nc.vector.reciprocal(var[:], var[:])
                nc.vector.tensor_mul(scale_t[l][:, b, :], g_sb[l][:, b, :], var[:])
                nc.vector.tensor_mul(tmp[:], mu[:], scale_t[l][:, b, :])
                nc.vector.tensor_sub(shift_t[l][:, b, :], be_sb[l][:, b, :], tmp[:])

            # ================= passes 1+2 (share the resident xT) ========
            with tc.tile_pool(name="xtp", bufs=1) as xtp:
              # resident transposed input, bf16: [feat, branch, node]
              xT = xtp.tile([128, 2, NP], BF16, name="xT")

              # ---- pass 1: transpose + Gram stats ----
              with (
                tc.tile_pool(name="w1p", bufs=2) as w1p,
                tc.tile_pool(name="ps_pt", bufs=5, space="PSUM") as ps_pt,
                tc.tile_pool(name="ps_g", bufs=1, space="PSUM") as ps_g,
                tc.tile_pool(name="ps_pj", bufs=1, space="PSUM") as ps_pj,
              ):
                  for b in range(2):
                      g_ps = ps_g.tile([128, 128], F32, tag="G", name=f"G_{b}")
                      nsub = NP // 128  # 98
                      si = 0
                      for c, (c0, tc_sz) in enumerate(SCH):
                          xin = w1p.tile([128, tc_sz // 128, 128], F32R, tag="xin",
                                         name=f"xin_{c}_{b}")
                          nc.sync.dma_start(
                              xin[:],
                              xd[b][c0:c0 + tc_sz, :].rearrange(
                                  "(j p) f -> p j f", p=128))
                          xin_bf = w1p.tile([128, tc_sz // 128, 128], BF16,
                                            tag="xinbf", name=f"xinbf_{c}_{b}")
                          nc.vector.tensor_copy(xin_bf[:], xin[:])
                          for j in range(tc_sz // 128):
                              pt = ps_pt.tile([128, 128], F32R, tag="pt",
                                              name=f"pt_{c}_{b}_{j}")
                              nc.tensor.transpose(pt[:], xin[:, j, :], identr)
                              if j % 2 == 0:
                                  nc.scalar.copy(
                                      xT[:, b, c0 + j * 128:c0 + (j + 1) * 128],
                                      pt[:])
                              else:
                                  nc.vector.tensor_copy(
                                      xT[:, b, c0 + j * 128:c0 + (j + 1) * 128],
                                      pt[:])
                              nc.tensor.matmul(g_ps[:], xin_bf[:, j, :],
                                               xin_bf[:, j, :],
                                               start=(si == 0), stop=(si == nsub - 1))
                              si += 1
                          nc.vector.reduce_sum(
                              xs_sl[:, b, c // 1:c // 1 + 1], xT[:, b, c0:c0 + tc_sz],
                              axis=mybir.AxisListType.X)
                      if b == 0:
                          load_pass2_weights()
                      # ---- project Gram -> (sum, sumsq) of pre1 ----
                      g_sbuf = w1p.tile([128, 128], F32R, tag="gsb", name=f"gsb_{b}")
                      nc.vector.tensor_copy(g_sbuf[:], g_ps[:])
                      mm1 = ps_pj.tile([128, 512], F32, tag="pj", name=f"mm1_{b}")
                      nc.tensor.matmul(mm1[:], g_sbuf[:], w1_r[b][:], start=True,
                                       stop=True)
                      mm1_sb = w1p.tile([128, 512], F32R, tag="mm1sb",
                                        name=f"mm1sb_{b}")
                      nc.vector.tensor_copy(mm1_sb[:], mm1[:])
                      prod = w1p.tile([128, 512], F32R, tag="prod", name=f"prod_{b}")
                      nc.vector.tensor_mul(prod[:], w1_r[b][:], mm1_sb[:])
                      xsum = w1p.tile([128, 1], F32, tag="xsum", name=f"xsum_{b}")
                      nc.vector.reduce_sum(xsum[:], xs_sl[:, b, :],
                                           axis=mybir.AxisListType.X)
                      xsum_r = w1p.tile([128, 4], F32R, tag="xsumr",
                                        name=f"xsumr_{b}")
                      for q in range(4):
                          nc.vector.tensor_copy(xsum_r[:, q:q + 1], xsum[:])
                      for m in range(4):
                          sq = ps_pj.tile([128, 4], F32, tag="pj2", name=f"sq_{b}_{m}")
                          nc.tensor.matmul(sq[:], prod[:, ts(m, 128)], ones_r,
                                           start=True, stop=True)
                          nc.vector.tensor_copy(pay[0][b][:, m, 1:2], sq[:, 0:1])
                          sm = ps_pj.tile([128, 4], F32, tag="pj2", name=f"sm_{b}_{m}")
                          nc.tensor.matmul(sm[:], w1_r[b][:, ts(m, 128)], xsum_r[:],
                                           start=True, stop=True)
                          nc.vector.tensor_copy(pay[0][b][:, m, 0:1], sm[:, 0:1])
                      issue_allreduce(0, b)

              # ================= pass 2: L1 -> BN1 -> L2 -> stats/spill ====
              with (
                  tc.tile_pool(name="w2p", bufs=4) as w2p,
                  tc.tile_pool(name="ps_p1", bufs=3, space="PSUM") as ps_p1,
                  tc.tile_pool(name="ps_p2", bufs=2, space="PSUM") as ps_p2,
              ):
                  for b in range(2):
                      finish_stats(0, b)
                      for c, (c0, tc_sz) in enumerate(CHUNKS):
                          h1 = w2p.tile([128, 4, tc_sz], F32R, tag="h1",
                                        name=f"h1_{c}_{b}")
                          for m in range(4):
                              pp = ps_p1.tile([128, tc_sz], F32, tag="p1",
                                              name=f"p1_{c}_{b}_{m}")
                              nc.tensor.matmul(pp[:], w1_bf[b][:, ts(m, 128)],
                                               xT[:, b, c0:c0 + tc_sz],
                                               start=True, stop=True)
                              nc.scalar.activation(
                                  h1[:, m, :], pp[:], AF.Relu,
                                  bias=shift_t[0][:, b, m:m + 1],
                                  scale=scale_t[0][:, b, m:m + 1])
                          if c == C - 1:
                              # padded nodes: relu(shift) != 0 would pollute BN2 stats
                              nc.scalar.mul(h1[:, :, PAD0:], h1[:, :, PAD0:], 0.0)
                          spl = w2p.tile([128, 4, tc_sz], BF16, tag="spl",
                                         name=f"spl_{c}_{b}")
                          for m in range(4):
                              pq = ps_p2.tile([128, tc_sz], F32, tag="p2",
                                              name=f"p2_{c}_{b}_{m}")
                              for k in range(4):
                                  nc.tensor.matmul(pq[:], w2_t[b][:, k, ts(m, 128)],
                                                   h1[:, k, :],
                                                   start=(k == 0), stop=(k == 3))
                              # copy frees the PSUM bank; stats read the SBUF
                              # copy and can lag without stalling the PE
                              if m % 2 == 0:
                                  nc.scalar.copy(spl[:, m, :], pq[:])
                              else:
                                  nc.vector.tensor_copy(spl[:, m, :], pq[:])
                          for m in range(4):
                              nc.vector.bn_stats(st2[:, b, m, c, :], spl[:, m, :])
                          nc.sync.dma_start(spill[:, b, :, c0:c0 + tc_sz], spl[:])
                      # ---- aggregate local BN2 stats, launch AllReduce ----
                      agg = stat.tile([128, 4, 2], F32, tag="agg", name=f"agg_{b}")
                      for m in range(4):
                          nc.vector.bn_aggr(agg[:, m, :], st2[:, b, m, :, :])
                      tmp2 = stat.tile([128, 4], F32, tag="tmp2", name=f"tmp2_{b}")
                      nc.vector.tensor_scalar_mul(pay[1][b][:, :, 0], agg[:, :, 0],
                                                  float(NP))
                      nc.vector.tensor_mul(tmp2[:], agg[:, :, 0], agg[:, :, 0])
                      nc.vector.tensor_add(tmp2[:], tmp2[:], agg[:, :, 1])
                      nc.vector.tensor_scalar_mul(pay[1][b][:, :, 1], tmp2[:],
                                                  float(NP))
                      issue_allreduce(1, b)
                      if b == 0:
                          load_pass3_weights()
                          # prefetch first b0 spill chunks on the gpsimd
                          # SWDGE queue (drains right after AR(1,0)), so
                          # pass-3 b0 sections can run during AR(1,1)
                          for cq in range(PF3):
                              cq0, cqs = CHUNKS[cq]
                              pftile = pf3.tile([128, 4, cqs], BF16, tag="pf",
                                                name=f"pf3_{cq}")
                              nc.gpsimd.dma_start(
                                  pftile[:], spill[:, 0, :, cq0:cq0 + cqs])
                              pf3_tiles[cq] = pftile
                          load_stats(1, 0, dma_engine=nc.gpsimd)

            # ================= pass 3: BN2 -> Lf -> head =================
            with (
                tc.tile_pool(name="w3p", bufs=2) as w3p,
                tc.tile_pool(name="ps_f", bufs=3, space="PSUM") as ps_f,
                tc.tile_pool(name="ps_t", bufs=3, space="PSUM") as ps_t,
            ):
                ps_o = ps_t  # share the 2 t/o banks (tag-separated slots share pool)
                finish_stats(1, 0)

                f_tiles = {}

                def b0_section(c):
                    c0, tc_sz = CHUNKS[c]
                    f_sb = w3p.tile([128, 4, tc_sz], F32R, tag="f0_sb", bufs=6,
                                    name=f"f0_sb_{c}")
                    f_tiles[c] = f_sb
                    if c in pf3_tiles:
                        pre2 = pf3_tiles.pop(c)
                    else:
                        pre2 = w3p.tile([128, 4, tc_sz], BF16, tag="pre2ld",
                                        bufs=4, name=f"pre2_{c}_0")
                        nc.sync.dma_start(pre2[:], spill[:, 0, :, c0:c0 + tc_sz])
                    h2 = w3p.tile([128, 4, tc_sz], F32R, tag="h2", bufs=3,
                                  name=f"h2_{c}_0")
                    for k in range(4):
                        nc.scalar.activation(
                            h2[:, k, :], pre2[:, k, :], AF.Relu,
                            bias=shift_t[1][:, 0, k:k + 1],
                            scale=scale_t[1][:, 0, k:k + 1])
                    for m in range(4):
                        pf = ps_f.tile([128, tc_sz], F32, tag="f",
                                       name=f"pf_{c}_0_{m}")
                        for k in range(4):
                            nc.tensor.matmul(pf[:], wf_t[0][:, k, ts(m, 128)],
                                             h2[:, k, :],
                                             start=(k == 0), stop=(k == 3))
                        nc.vector.tensor_scalar_add(
                            f_sb[:, m, :], pf[:], bf_sb[0][:, m:m + 1])

                # run-ahead: b0 sections of the prefetched chunks execute
                # while AllReduce (1,1) is still in flight
                for c in range(PF3):
                    b0_section(c)
                finish_stats(1, 1)

                for c, (c0, tc_sz) in enumerate(CHUNKS):
                    if c >= PF3:
                        b0_section(c)
                    f0_sb = f_tiles.pop(c)
                    f1_sb = w3p.tile([128, 4, tc_sz], F32R, tag="f1_sb", bufs=2,
                                     name=f"f1_sb_{c}")
                    # ---- branch 1 section ----
                    pre2 = w3p.tile([128, 4, tc_sz], BF16, tag="pre2ld",
                                    bufs=4, name=f"pre2_{c}_1")
                    nc.sync.dma_start(pre2[:], spill[:, 1, :, c0:c0 + tc_sz])
                    h2 = w3p.tile([128, 4, tc_sz], F32R, tag="h2", bufs=3,
                                  name=f"h2_{c}_1")
                    for k in range(4):
                        nc.scalar.activation(
                            h2[:, k, :], pre2[:, k, :], AF.Relu,
                            bias=shift_t[1][:, 1, k:k + 1],
                            scale=scale_t[1][:, 1, k:k + 1])
                    for m in range(4):
                        pf = ps_f.tile([128, tc_sz], F32, tag="f",
                                       name=f"pf_{c}_1_{m}")
                        for k in range(4):
                            nc.tensor.matmul(pf[:], wf_t[1][:, k, ts(m, 128)],
                                             h2[:, k, :],
                                             start=(k == 0), stop=(k == 3))
                        nc.vector.tensor_scalar_add(
                            f1_sb[:, m, :], pf[:], bf_sb[1][:, m:m + 1])
                    # ---- head ----
                    t_sb = w3p.tile([128, 4, tc_sz], F32R, tag="t_sb", bufs=1,
                                    name=f"t_sb_{c}")
                    for m in range(4):
                        ptl = ps_t.tile([128, tc_sz], F32, tag="t",
                                        name=f"ptl_{c}_{m}")
                        for k in range(8):
                            fsrc = f0_sb[:, k, :] if k < 4 else f1_sb[:, k - 4, :]
                            nc.tensor.matmul(ptl[:], wh1_t[:, k, ts(m, 128)],
                                             fsrc,
                                             start=(k == 0), stop=(k == 7))
                        nc.scalar.activation(t_sb[:, m, :], ptl[:], AF.Relu,
                                             bias=bh1_sb[:, m:m + 1])
                    po = ps_o.tile([10, tc_sz], F32, tag="t", name=f"po_{c}")
                    for k in range(4):
                        nc.tensor.matmul(po[:], wh2_t[:, k, :], t_sb[:, k, :],
                                         start=(k == 0), stop=(k == 3))
                    o_sb = w3p.tile([10, tc_sz], F32, tag="o_sb", name=f"o_sb_{c}")
                    nc.scalar.activation(o_sb[:], po[:], AF.Identity,
                                         bias=bh2_sb[:, 0:1])
                    nc.sync.dma_start(outd[:, c0:c0 + tc_sz], o_sb[:])

    nc.compile()
    return nc


def _get_program():
    if "nc" not in _CACHE:
        _CACHE["nc"] = _build_program()
    return _CACHE["nc"]


def kernel(**inputs):
    nc = _get_program()

    def shard_x(x):
        x = np.ascontiguousarray(x, dtype=np.float32).reshape(NCORES, NSH, 128)
        pad = np.zeros((NCORES, NP - NSH, 128), dtype=np.float32)
        return np.concatenate([x, pad], axis=1)

    xs = [shard_x(inputs["x_1"]), shard_x(inputs["x_2"])]
    rep = {}
    for nm in ("W1_1", "W2_1", "Wf_1", "bf_1", "g1_1", "be1_1", "g2_1", "be2_1",
               "W1_2", "W2_2", "Wf_2", "bf_2", "g1_2", "be1_2", "g2_2", "be2_2",
               "Wh1", "bh1", "Wh2", "bh2"):
        rep[nm] = np.ascontiguousarray(inputs[nm], dtype=np.float32)

    aux = np.zeros((128, 132), dtype=np.float32)
    aux[:, :128] = np.eye(128, dtype=np.float32)
    aux[:, 128:132] = 1.0
    rep["AUX"] = aux
    rep["EPSA"] = np.full((128, 1), EPS, dtype=np.float32)

    in_maps = []
    for c in range(NCORES):
        m = {"x_1": xs[0][c], "x_2": xs[1][c]}
        m.update(rep)
        in_maps.append(m)

    res = bass_utils.run_bass_kernel_spmd(nc, in_maps, core_ids=list(range(NCORES)))
    parts = [res.results[c]["OUT"][:, :NSH] for c in range(NCORES)]
    out = np.concatenate(parts, axis=1).T
    return np.ascontiguousarray(out, dtype=np.float32)



# revision 4
# speedup vs baseline: 1.1959x; 1.1959x over previous
"""ChebyNet (K=1) dual-branch MLP + BN kernel for 8 Trainium2 NeuronCores.

Network (per reference):
  branch b in {1,2}:  h = relu(BN(x_b @ W1_b)) ; h = relu(BN(h @ W2_b)) ; f_b = h @ Wf_b + bf_b
  out = relu(concat(f_1, f_2) @ Wh1 + bh1) @ Wh2 + bh2

ChebConv with K=1 ignores edge_index/edge_weight entirely.  Training-mode
BatchNorm over the node axis makes the linear-layer biases b1/b2 cancel
exactly, so they are never loaded.

Key restructurings vs the direct form:
  * Wf_b and Wh1 compose linearly (no nonlinearity between them), so the
    host folds M_b = Wf_b @ Wh1[b-half] and b' = bf_1 @ Wh1a + bf_2 @ Wh1b
    + bh1.  The Lf layer and the concat disappear: t = relu(h2_1 @ M_1 +
    h2_2 @ M_2 + b'), out = t @ Wh2 + bh2.
  * The host pre-transposes x into feature-major xT (bf16) and a
    partition-major node layout xg for the Gram pass, so the kernel never
    runs PE transposes.
  * Layer-1 BN stats use the Gram identity: sumsq(pre1) = diag(W1^T (X^T X)
    W1), sum(pre1) = W1^T (X^T 1).  X^T 1 comes from the same Gram
    stationary tiles with a [128,4] ones moving operand (nearly free).
  * Pass 2 computes branch-0 L1 for all chunks before BN1 stats arrive,
    parking pre1 as bf16 ("loop A"); the BN1+relu+L2 pass ("loop B") then
    never stalls on the first AllReduce.

Sharding: nodes (axis 0) split across 8 cores, 12500 each, zero-padded to
12544 = 98*128.  Weights replicated.  BN batch stats are combined with an
AllReduce(add) of per-core (sum, sumsq); the four collectives are
interleaved so each hides under the next phase's compute.
"""

import os

os.environ.setdefault("JAX_PLATFORMS", "axon,cpu")

import numpy as np

import concourse.bacc as bacc
import concourse.mybir as mybir
import concourse.tile as tile
from concourse import bass_utils
from concourse.bass import ts

F32 = mybir.dt.float32
F32R = mybir.dt.float32r
BF16 = mybir.dt.bfloat16
AF = mybir.ActivationFunctionType
ALU = mybir.AluOpType

NTOT = 100000          # true node count
NCORES = 8
NSH = NTOT // NCORES   # 12500 true nodes per core
NP = 12544             # padded per-core nodes (= 98 * 128)
T = 512                # node-chunk size (free dim of matmuls / PSUM bank)
CHUNKS = [(i * T, T) for i in range(NP // T)] + ([(NP - NP % T, NP % T)] if NP % T else [])
C = len(CHUNKS)
NSUB = NP // 128       # 98 gram sub-tiles
GGRP = 8               # gram sub-tiles per DMA
GRAM_GROUPS = [(i * GGRP, GGRP) for i in range(NSUB // GGRP)]
if NSUB % GGRP:
    GRAM_GROUPS.append((NSUB - NSUB % GGRP, NSUB % GGRP))
PAD0 = NSH - (NP - (NP % T or T))  # first padded column inside last chunk (212)
EPS = 1e-5

_CACHE = {}


def _build_program():
    nc = bacc.Bacc("TRN2", target_bir_lowering=False, debug=False,
                   num_devices=NCORES)

    # ---- kernel I/O -----------------------------------------------------
    xT_d = [nc.dram_tensor(f"xT_{b}", [128, NP], BF16, kind="ExternalInput")
            for b in range(2)]
    xg_d = [nc.dram_tensor(f"xg_{b}", [128, NSUB, 128], BF16,
                           kind="ExternalInput") for b in range(2)]
    w1_d = [nc.dram_tensor(f"W1_{b}", [128, 512], BF16, kind="ExternalInput")
            for b in range(2)]
    w2_d = [nc.dram_tensor(f"W2_{b}", [128, 4, 512], F32R,
                           kind="ExternalInput") for b in range(2)]
    m_d = [nc.dram_tensor(f"M_{b}", [128, 4, 512], BF16, kind="ExternalInput")
           for b in range(2)]
    wh2_d = nc.dram_tensor("WH2", [128, 4, 10], BF16, kind="ExternalInput")
    bp_d = nc.dram_tensor("BP", [128, 4], F32, kind="ExternalInput")
    bh2_d = nc.dram_tensor("BH2", [10, 1], F32, kind="ExternalInput")
    g_d = [[nc.dram_tensor(f"g{l}_{b}", [128, 4], F32, kind="ExternalInput")
            for b in range(2)] for l in range(2)]
    be_d = [[nc.dram_tensor(f"be{l}_{b}", [128, 4], F32, kind="ExternalInput")
             for b in range(2)] for l in range(2)]
    outd = nc.dram_tensor("OUT", [10, NP], F32, kind="ExternalOutput")

    # ---- DRAM scratch ---------------------------------------------------
    spill = nc.dram_tensor("pre2_spill", [128, 2, 4, NP], BF16)
    cc_in = [[nc.dram_tensor(f"cc{l}{b}_in", [128, 4, 2], F32) for b in range(2)]
             for l in range(2)]
    cc_out = [[nc.dram_tensor(f"cc{l}{b}_out", [128, 4, 2], F32,
                              addr_space="Shared") for b in range(2)]
              for l in range(2)]

    with tile.TileContext(nc) as tc:
        with (
            tc.tile_pool(name="wpool", bufs=1) as wp,
            tc.tile_pool(name="stat", bufs=1) as stat,
        ):
            # ---- constants via cheap DVE memsets -------------------------
            ones_bf = wp.tile([128, 4], BF16, name="ones_bf")
            nc.vector.memset(ones_bf[:], 1.0)
            ones_r = wp.tile([128, 4], F32R, name="ones_r")
            nc.vector.memset(ones_r[:], 1.0)
            eps_t = stat.tile([128, 1], F32, name="eps_t")
            nc.vector.memset(eps_t[:], EPS)

            # W1 now (pass-1 projection needs it); the rest deferred.
            w1_bf, w1_r = [], []
            for b in range(2):
                w1b = wp.tile([128, 512], BF16, name=f"w1b_{b}")
                nc.scalar.dma_start(w1b[:], w1_d[b][:, :])
                w1r = wp.tile([128, 512], F32R, name=f"w1r_{b}")
                nc.vector.tensor_copy(w1r[:], w1b[:])
                w1_bf.append(w1b)
                w1_r.append(w1r)

            w2_t = [wp.tile([128, 4, 512], F32R, name=f"w2_{b}") for b in range(2)]
            m_t = [wp.tile([128, 4, 512], BF16, name=f"m_{b}") for b in range(2)]
            wh2_t = wp.tile([128, 4, 10], BF16, name="wh2_t")
            bp_sb = wp.tile([128, 4], F32, name="bp_sb")
            bh2_sb = wp.tile([10, 1], F32, name="bh2_sb")
            g_sb = [stat.tile([128, 2, 4], F32, name=f"g_sb{l}") for l in range(2)]
            be_sb = [stat.tile([128, 2, 4], F32, name=f"be_sb{l}") for l in range(2)]

            def load_pass2_weights():
                for b in range(2):
                    nc.scalar.dma_start(w2_t[b][:], w2_d[b][:, :, :])
                for l in range(2):
                    for b in range(2):
                        nc.scalar.dma_start(g_sb[l][:, b, :], g_d[l][b][:, :])
                        nc.scalar.dma_start(be_sb[l][:, b, :], be_d[l][b][:, :])

            def load_pass3_weights():
                for b in range(2):
                    nc.scalar.dma_start(m_t[b][:], m_d[b][:, :, :])
                nc.scalar.dma_start(wh2_t[:], wh2_d[:, :, :])
                nc.scalar.dma_start(bp_sb[:], bp_d[:, :])
                nc.scalar.dma_start(bh2_sb[:], bh2_d[:, :])

            st2 = stat.tile([128, 2, 4, C, 6], F32, name="st2")
            pay = [[stat.tile([128, 4, 2], F32, name=f"pay{l}{b}")
                    for b in range(2)] for l in range(2)]
            scale_t = [stat.tile([128, 2, 4], F32, name=f"scale{l}") for l in range(2)]
            shift_t = [stat.tile([128, 2, 4], F32, name=f"shift{l}") for l in range(2)]

            def issue_allreduce(l, b):
                nc.sync.dma_start(cc_in[l][b][:, :, :], pay[l][b][:])
                nc.gpsimd.collective_compute(
                    "AllReduce", mybir.AluOpType.add,
                    replica_groups=[list(range(NCORES))],
                    ins=[cc_in[l][b].ap().opt()], outs=[cc_out[l][b].ap().opt()],
                )

            def finish_stats(l, b):
                """cc_out[l][b] -> scale_t[l][:, b, :], shift_t[l][:, b, :]."""
                gl = stat.tile([128, 4, 2], F32, tag="gl", name=f"gl{l}{b}")
                nc.sync.dma_start(gl[:], cc_out[l][b][:, :, :])
                mu = stat.tile([128, 4], F32, tag="mu", name=f"mu{l}{b}")
                var = stat.tile([128, 4], F32, tag="var", name=f"var{l}{b}")
                tmp = stat.tile([128, 4], F32, tag="tmpf", name=f"tmp{l}{b}")
                nc.vector.tensor_scalar_mul(mu[:], gl[:, :, 0], 1.0 / NTOT)
                nc.vector.tensor_scalar_mul(var[:], gl[:, :, 1], 1.0 / NTOT)
                nc.vector.tensor_mul(tmp[:], mu[:], mu[:])
                nc.vector.tensor_sub(var[:], var[:], tmp[:])
                nc.scalar.activation(var[:], var[:], AF.Sqrt, bias=eps_t[:])
                nc.vector.reciprocal(var[:], var[:])
                nc.vector.tensor_mul(scale_t[l][:, b, :], g_sb[l][:, b, :], var[:])
                nc.vector.tensor_mul(tmp[:], mu[:], scale_t[l][:, b, :])
                nc.vector.tensor_sub(shift_t[l][:, b, :], be_sb[l][:, b, :], tmp[:])

            # ================= pass 1: Gram + xsum -> BN1 stats ==========
            with (
                tc.tile_pool(name="xtp0", bufs=1) as xtp0,
                tc.tile_pool(name="parkp", bufs=1) as parkp,
                tc.tile_pool(name="w2p", bufs=2) as w2p,
            ):
              with (
                tc.tile_pool(name="g1p", bufs=1) as g1p,
                tc.tile_pool(name="ps_g", bufs=1, space="PSUM") as ps_g,
                tc.tile_pool(name="ps_xs", bufs=1, space="PSUM") as ps_xs,
                tc.tile_pool(name="ps_pj", bufs=1, space="PSUM") as ps_pj,
              ):
                def gram_pass(b):
                    g_ps = ps_g.tile([128, 128], F32, tag="G", name=f"G_{b}")
                    xs_ps = ps_xs.tile([128, 4], F32, tag="XS", name=f"XS_{b}")
                    si = 0
                    for j0, gsz in GRAM_GROUPS:
                        xgt = g1p.tile([128, GGRP, 128], BF16, tag="xg",
                                       name=f"xg_{b}_{j0}")
                        nc.sync.dma_start(xgt[:, :gsz, :],
                                          xg_d[b][:, j0:j0 + gsz, :])
                        for j in range(gsz):
                            nc.tensor.matmul(g_ps[:], xgt[:, j, :], xgt[:, j, :],
                                             start=(si == 0), stop=(si == NSUB - 1))
                            nc.tensor.matmul(xs_ps[:], xgt[:, j, :], ones_bf[:],
                                             start=(si == 0), stop=(si == NSUB - 1))
                            si += 1
                    return g_ps, xs_ps

                def proj_pass(b, g_ps, xs_ps):
                    g_sbuf = g1p.tile([128, 128], F32R, tag="gsb", name=f"gsb_{b}")
                    nc.vector.tensor_copy(g_sbuf[:], g_ps[:])
                    xsum_r = g1p.tile([128, 4], F32R, tag="xsumr", name=f"xsumr_{b}")
                    nc.vector.tensor_copy(xsum_r[:], xs_ps[:])
                    mm1 = ps_pj.tile([128, 512], F32, tag="pj", name=f"mm1_{b}")
                    nc.tensor.matmul(mm1[:], g_sbuf[:], w1_r[b][:], start=True,
                                     stop=True)
                    mm1_sb = g1p.tile([128, 512], F32R, tag="mm1sb",
                                      name=f"mm1sb_{b}")
                    nc.vector.tensor_copy(mm1_sb[:], mm1[:])
                    prod = g1p.tile([128, 512], F32R, tag="prod", name=f"prod_{b}")
                    nc.vector.tensor_mul(prod[:], w1_r[b][:], mm1_sb[:])
                    for m in range(4):
                        sq = ps_pj.tile([128, 4], F32, tag="pj2", name=f"sq_{b}_{m}")
                        nc.tensor.matmul(sq[:], prod[:, ts(m, 128)], ones_r[:],
                                         start=True, stop=True)
                        nc.vector.tensor_copy(pay[0][b][:, m, 1:2], sq[:, 0:1])
                        sm = ps_pj.tile([128, 4], F32, tag="pj2", name=f"sm_{b}_{m}")
                        nc.tensor.matmul(sm[:], w1_r[b][:, ts(m, 128)], xsum_r[:],
                                         start=True, stop=True)
                        nc.vector.tensor_copy(pay[0][b][:, m, 0:1], sm[:, 0:1])
                    issue_allreduce(0, b)

                g0, xs0 = gram_pass(0)
                proj_pass(0, g0, xs0)

                # pass-2 b0 input loads (stats-independent, early for DMA
                # ordering: interleave ahead of branch-1 gram loads)
                xt0_tiles = {}
                for c, (c0, tc_sz) in enumerate(CHUNKS):
                    xt = xtp0.tile([128, tc_sz], BF16, tag=f"xt0_{c}",
                                   name=f"xt0_{c}")
                    nc.sync.dma_start(xt[:], xT_d[0][:, c0:c0 + tc_sz])
                    xt0_tiles[c] = xt

                g1, xs1 = gram_pass(1)
                proj_pass(1, g1, xs1)
                load_pass2_weights()

              with (
                tc.tile_pool(name="ps_p1", bufs=3, space="PSUM") as ps_p1,
                tc.tile_pool(name="ps_p2", bufs=3, space="PSUM") as ps_p2,
              ):
                # ---- pass-2 b0 loop A: L1 + park (stats-independent) ----
                park_tiles = {}
                for c, (c0, tc_sz) in enumerate(CHUNKS):
                    park = parkp.tile([128, 4, tc_sz], BF16, tag=f"park_{c}",
                                      name=f"park_{c}")
                    park_tiles[c] = park
                    for m in range(4):
                        pp = ps_p1.tile([128, tc_sz], F32, tag="p1",
                                        name=f"p1a_{c}_{m}")
                        nc.tensor.matmul(pp[:], w1_bf[0][:, ts(m, 128)],
                                         xt0_tiles[c][:], start=True, stop=True)
                        if m % 2 == 0:
                            nc.scalar.copy(park[:, m, :], pp[:])
                        else:
                            nc.vector.tensor_copy(park[:, m, :], pp[:])

                # ---- pass-2 b0 loop B: BN1 -> L2 -> stats/spill ---------
                finish_stats(0, 0)
                for c, (c0, tc_sz) in enumerate(CHUNKS):
                    park = park_tiles.pop(c)
                    h1 = w2p.tile([128, 4, tc_sz], F32R, tag="h1",
                                  name=f"h1_{c}_0")
                    for m in range(4):
                        nc.scalar.activation(
                            h1[:, m, :], park[:, m, :], AF.Relu,
                            bias=shift_t[0][:, 0, m:m + 1],
                            scale=scale_t[0][:, 0, m:m + 1])
                    if c == C - 1:
                        nc.scalar.mul(h1[:, :, PAD0:], h1[:, :, PAD0:], 0.0)
                    spl = w2p.tile([128, 4, tc_sz], BF16, tag="spl",
                                   name=f"spl_{c}_0", bufs=4)
                    for m in range(4):
                        pq = ps_p2.tile([128, tc_sz], F32, tag="p2",
                                        name=f"p2_{c}_0_{m}")
                        for k in range(4):
                            nc.tensor.matmul(pq[:], w2_t[0][:, k, ts(m, 128)],
                                             h1[:, k, :],
                                             start=(k == 0), stop=(k == 3))
                        if m % 2 == 0:
                            nc.scalar.copy(spl[:, m, :], pq[:])
                        else:
                            nc.vector.tensor_copy(spl[:, m, :], pq[:])
                    for m in range(4):
                        nc.vector.bn_stats(st2[:, 0, m, c, :], spl[:, m, :])
                    nc.sync.dma_start(spill[:, 0, :, c0:c0 + tc_sz], spl[:])

                agg = stat.tile([128, 4, 2], F32, tag="agg", name="agg_0")
                for m in range(4):
                    nc.vector.bn_aggr(agg[:, m, :], st2[:, 0, m, :, :])
                tmp2 = stat.tile([128, 4], F32, tag="tmp2", name="tmp2_0")
                nc.vector.tensor_scalar_mul(pay[1][0][:, :, 0], agg[:, :, 0],
                                            float(NP))
                nc.vector.tensor_mul(tmp2[:], agg[:, :, 0], agg[:, :, 0])
                nc.vector.tensor_add(tmp2[:], tmp2[:], agg[:, :, 1])
                nc.vector.tensor_scalar_mul(pay[1][0][:, :, 1], tmp2[:],
                                            float(NP))
                issue_allreduce(1, 0)

                # ---- pass-2 b1 (direct PSUM path; stats(0,1) long ready) -
                finish_stats(0, 1)
                load_pass3_weights()
                for c, (c0, tc_sz) in enumerate(CHUNKS):
                    xt = xtp0.tile([128, tc_sz], BF16, tag="xt1",
                                   name=f"xt1_{c}", bufs=8)
                    nc.sync.dma_start(xt[:], xT_d[1][:, c0:c0 + tc_sz])
                    h1 = w2p.tile([128, 4, tc_sz], F32R, tag="h1",
                                  name=f"h1_{c}_1")
                    for m in range(4):
                        pp = ps_p1.tile([128, tc_sz], F32, tag="p1",
                                        name=f"p1b_{c}_{m}")
                        nc.tensor.matmul(pp[:], w1_bf[1][:, ts(m, 128)],
                                         xt[:], start=True, stop=True)
                        nc.scalar.activation(
                            h1[:, m, :], pp[:], AF.Relu,
                            bias=shift_t[0][:, 1, m:m + 1],
                            scale=scale_t[0][:, 1, m:m + 1])
                    if c == C - 1:
                        nc.scalar.mul(h1[:, :, PAD0:], h1[:, :, PAD0:], 0.0)
                    spl = w2p.tile([128, 4, tc_sz], BF16, tag="spl",
                                   name=f"spl_{c}_1", bufs=4)
                    for m in range(4):
                        pq = ps_p2.tile([128, tc_sz], F32, tag="p2",
                                        name=f"p2_{c}_1_{m}")
                        for k in range(4):
                            nc.tensor.matmul(pq[:], w2_t[1][:, k, ts(m, 128)],
                                             h1[:, k, :],
                                             start=(k == 0), stop=(k == 3))
                        if m % 2 == 0:
                            nc.scalar.copy(spl[:, m, :], pq[:])
                        else:
                            nc.vector.tensor_copy(spl[:, m, :], pq[:])
                    for m in range(4):
                        nc.vector.bn_stats(st2[:, 1, m, c, :], spl[:, m, :])
                    nc.sync.dma_start(spill[:, 1, :, c0:c0 + tc_sz], spl[:])

                agg1 = stat.tile([128, 4, 2], F32, tag="agg", name="agg_1")
                for m in range(4):
                    nc.vector.bn_aggr(agg1[:, m, :], st2[:, 1, m, :, :])
                tmp21 = stat.tile([128, 4], F32, tag="tmp2", name="tmp2_1")
                nc.vector.tensor_scalar_mul(pay[1][1][:, :, 0], agg1[:, :, 0],
                                            float(NP))
                nc.vector.tensor_mul(tmp21[:], agg1[:, :, 0], agg1[:, :, 0])
                nc.vector.tensor_add(tmp21[:], tmp21[:], agg1[:, :, 1])
                nc.vector.tensor_scalar_mul(pay[1][1][:, :, 1], tmp21[:],
                                            float(NP))
                issue_allreduce(1, 1)

            # ================= pass 3: BN2 -> fused M -> head ============
            with (
                tc.tile_pool(name="w3p", bufs=2) as w3p,
                tc.tile_pool(name="ps_t", bufs=6, space="PSUM") as ps_t,
                tc.tile_pool(name="ps_o", bufs=2, space="PSUM") as ps_o,
            ):
                finish_stats(1, 0)
                finish_stats(1, 1)

                for c, (c0, tc_sz) in enumerate(CHUNKS):
                    pre2 = [w3p.tile([128, 4, tc_sz], BF16, tag=f"pre2_{b}",
                                     bufs=3, name=f"pre2_{c}_{b}")
                            for b in range(2)]
                    for b in range(2):
                        nc.sync.dma_start(pre2[b][:],
                                          spill[:, b, :, c0:c0 + tc_sz])
                    # branch-0 h2 on ACT
                    h2_0 = w3p.tile([128, 4, tc_sz], BF16, tag="h2_0",
                                    bufs=3, name=f"h2_{c}_0")
                    for k in range(4):
                        nc.scalar.activation(
                            h2_0[:, k, :], pre2[0][:, k, :], AF.Relu,
                            bias=shift_t[1][:, 0, k:k + 1],
                            scale=scale_t[1][:, 0, k:k + 1])
                    # branch-1 h2 on DVE (2-op affine + relu, bf16 2x mode)
                    h2_1 = w3p.tile([128, 4, tc_sz], BF16, tag="h2_1",
                                    bufs=3, name=f"h2_{c}_1")
                    for k in range(4):
                        nc.vector.tensor_scalar(
                            h2_1[:, k, :], pre2[1][:, k, :],
                            scale_t[1][:, 1, k:k + 1],
                            shift_t[1][:, 1, k:k + 1],
                            ALU.mult, ALU.add)
                        nc.vector.tensor_scalar_max(h2_1[:, k, :],
                                                    h2_1[:, k, :], 0.0)
                    t_sb = w3p.tile([128, 4, tc_sz], BF16, tag="t_sb", bufs=2,
                                    name=f"t_sb_{c}")
                    for m in range(4):
                        ptl = ps_t.tile([128, tc_sz], F32, tag="t",
                                        name=f"ptl_{c}_{m}")
                        for k in range(4):
                            nc.tensor.matmul(ptl[:], m_t[0][:, k, ts(m, 128)],
                                             h2_0[:, k, :],
                                             start=(k == 0), stop=False)
                        for k in range(4):
                            nc.tensor.matmul(ptl[:], m_t[1][:, k, ts(m, 128)],
                                             h2_1[:, k, :],
                                             start=False, stop=(k == 3))
                        nc.scalar.activation(t_sb[:, m, :], ptl[:], AF.Relu,
                                             bias=bp_sb[:, m:m + 1])
                    po = ps_o.tile([10, tc_sz], F32, tag="o", name=f"po_{c}")
                    for k in range(4):
                        nc.tensor.matmul(po[:], wh2_t[:, k, :], t_sb[:, k, :],
                                         start=(k == 0), stop=(k == 3))
                    o_sb = w3p.tile([10, tc_sz], F32, tag="o_sb", name=f"o_sb_{c}")
                    nc.scalar.activation(o_sb[:], po[:], AF.Identity,
                                         bias=bh2_sb[:, 0:1])
                    nc.sync.dma_start(outd[:, c0:c0 + tc_sz], o_sb[:])

    nc.compile()
    return nc


def _get_program():
    if "nc" not in _CACHE:
        _CACHE["nc"] = _build_program()
    return _CACHE["nc"]


def kernel(**inputs):
    import ml_dtypes

    nc = _get_program()
    bf16 = ml_dtypes.bfloat16

    def shard_x(x):
        x = np.ascontiguousarray(x, dtype=np.float32).reshape(NCORES, NSH, 128)
        pad = np.zeros((NCORES, NP - NSH, 128), dtype=np.float32)
        return np.concatenate([x, pad], axis=1)  # [NCORES, NP, 128]

    xs = [shard_x(inputs["x_1"]), shard_x(inputs["x_2"])]
    # feature-major + gram layouts, bf16
    xT = [np.ascontiguousarray(x.transpose(0, 2, 1)).astype(bf16) for x in xs]
    xg = [np.ascontiguousarray(x.reshape(NCORES, 128, NSUB, 128)).astype(bf16)
          for x in xs]

    def km(w):  # [512, O] -> [128, 4, O] (contraction-major for lhsT slabs)
        O = w.shape[1]
        return np.ascontiguousarray(
            w.reshape(4, 128, O).transpose(1, 0, 2))

    def vec(v):  # [512] -> [128, 4]
        return np.ascontiguousarray(v.reshape(4, 128).T)

    f64 = np.float64
    Wh1 = np.asarray(inputs["Wh1"], f64)
    rep = {}
    for b, sfx in ((0, "1"), (1, "2")):
        rep[f"W1_{b}"] = np.asarray(inputs[f"W1_{sfx}"], np.float32).astype(bf16)
        rep[f"W2_{b}"] = km(np.asarray(inputs[f"W2_{sfx}"], np.float32))
        M = np.asarray(inputs[f"Wf_{sfx}"], f64) @ Wh1[b * 512:(b + 1) * 512, :]
        rep[f"M_{b}"] = km(M.astype(np.float32)).astype(bf16)
        for l, nm in ((0, "1"), (1, "2")):
            rep[f"g{l}_{b}"] = vec(np.asarray(inputs[f"g{nm}_{sfx}"], np.float32))
            rep[f"be{l}_{b}"] = vec(np.asarray(inputs[f"be{nm}_{sfx}"], np.float32))
    bp = (np.asarray(inputs["bf_1"], f64) @ Wh1[:512, :]
          + np.asarray(inputs["bf_2"], f64) @ Wh1[512:, :]
          + np.asarray(inputs["bh1"], f64))
    rep["BP"] = vec(bp.astype(np.float32))
    rep["WH2"] = km(np.asarray(inputs["Wh2"], np.float32)).astype(bf16)
    rep["BH2"] = np.ascontiguousarray(
        np.asarray(inputs["bh2"], np.float32).reshape(10, 1))

    in_maps = []
    for c in range(NCORES):
        m = {"xT_0": xT[0][c], "xT_1": xT[1][c],
             "xg_0": xg[0][c], "xg_1": xg[1][c]}
        m.update(rep)
        in_maps.append(m)

    res = bass_utils.run_bass_kernel_spmd(nc, in_maps, core_ids=list(range(NCORES)))
    parts = [res.results[c]["OUT"][:, :NSH] for c in range(NCORES)]
    out = np.concatenate(parts, axis=1).T
    return np.ascontiguousarray(out, dtype=np.float32)


# revision 22
# speedup vs baseline: 1.4235x; 1.1903x over previous
"""ChebyNet (K=1) dual-branch MLP + BN kernel for 8 Trainium2 NeuronCores.

Network (per reference):
  branch b in {1,2}:  h = relu(BN(x_b @ W1_b)) ; h = relu(BN(h @ W2_b)) ; f_b = h @ Wf_b + bf_b
  out = relu(concat(f_1, f_2) @ Wh1 + bh1) @ Wh2 + bh2

ChebConv with K=1 ignores edge_index/edge_weight entirely.  Training-mode
BatchNorm over the node axis makes the linear-layer biases b1/b2 cancel
exactly, so they are never loaded.

Key restructurings vs the direct form:
  * Wf_b and Wh1 compose linearly (no nonlinearity between them), so the
    host folds M_b = Wf_b @ Wh1[b-half] and b' = bf_1 @ Wh1a + bf_2 @ Wh1b
    + bh1.  The Lf layer and the concat disappear: t = relu(h2_1 @ M_1 +
    h2_2 @ M_2 + b'), out = t @ Wh2 + bh2.
  * The host pre-transposes x into feature-major xT (bf16) and a
    partition-major node layout xg for the Gram pass, so the kernel never
    runs PE transposes.
  * Layer-1 BN stats use the Gram identity: sumsq(pre1) = diag(W1^T (X^T X)
    W1), sum(pre1) = W1^T (X^T 1).  X^T 1 comes from the same Gram
    stationary tiles with a [128,4] ones moving operand (nearly free).
  * Pass 2 computes branch-0 L1 for all chunks before BN1 stats arrive,
    parking pre1 as bf16 ("loop A"); the BN1+relu+L2 pass ("loop B") then
    never stalls on the first AllReduce.

Sharding: nodes (axis 0) split across 8 cores, 12500 each, zero-padded to
12544 = 98*128.  Weights replicated.  BN batch stats are combined with an
AllReduce(add) of per-core (sum, sumsq); the four collectives are
interleaved so each hides under the next phase's compute.
"""

import os

os.environ.setdefault("JAX_PLATFORMS", "axon,cpu")

import numpy as np

import concourse.bacc as bacc
import concourse.mybir as mybir
import concourse.tile as tile
from concourse import bass_utils
from concourse.bass import ts

F32 = mybir.dt.float32
F32R = mybir.dt.float32r
BF16 = mybir.dt.bfloat16
AF = mybir.ActivationFunctionType
ALU = mybir.AluOpType

NTOT = 100000          # true node count
NCORES = 8
NSH = NTOT // NCORES   # 12500 true nodes per core
NP = 12544             # padded per-core nodes (= 98 * 128)
T = 512                # node-chunk size (free dim of matmuls / PSUM bank)
CHUNKS = [(i * T, T) for i in range(NP // T)] + ([(NP - NP % T, NP % T)] if NP % T else [])
C = len(CHUNKS)
NSUB = NP // 128       # 98 gram sub-tiles
GGRP = 8               # gram sub-tiles per DMA
GRAM_GROUPS = [(i * GGRP, GGRP) for i in range(NSUB // GGRP)]
if NSUB % GGRP:
    GRAM_GROUPS.append((NSUB - NSUB % GGRP, NSUB % GGRP))
PAD0 = NSH - (NP - (NP % T or T))  # first padded column inside last chunk (212)
EPS = 1e-5

_CACHE = {}


def _build_program():
    nc = bacc.Bacc("TRN2", target_bir_lowering=False, debug=False,
                   num_devices=NCORES)

    # ---- kernel I/O -----------------------------------------------------
    xT_d = [nc.dram_tensor(f"xT_{b}", [128, NP], BF16, kind="ExternalInput")
            for b in range(2)]
    xg_d = [nc.dram_tensor(f"xg_{b}", [128, NSUB, 128], BF16,
                           kind="ExternalInput") for b in range(2)]
    w1_d = [nc.dram_tensor(f"W1_{b}", [128, 512], BF16, kind="ExternalInput")
            for b in range(2)]
    w2_d = [nc.dram_tensor(f"W2_{b}", [128, 4, 512], F32R,
                           kind="ExternalInput") for b in range(2)]
    m_d = [nc.dram_tensor(f"M_{b}", [128, 4, 512], BF16, kind="ExternalInput")
           for b in range(2)]
    wh2_d = nc.dram_tensor("WH2", [128, 4, 10], BF16, kind="ExternalInput")
    bp_d = nc.dram_tensor("BP", [128, 4], F32, kind="ExternalInput")
    bh2_d = nc.dram_tensor("BH2", [10, 1], F32, kind="ExternalInput")
    g_d = [[nc.dram_tensor(f"g{l}_{b}", [128, 4], F32, kind="ExternalInput")
            for b in range(2)] for l in range(2)]
    be_d = [[nc.dram_tensor(f"be{l}_{b}", [128, 4], F32, kind="ExternalInput")
             for b in range(2)] for l in range(2)]
    outd = nc.dram_tensor("OUT", [10, NP], F32, kind="ExternalOutput")

    # ---- DRAM scratch ---------------------------------------------------
    spill = nc.dram_tensor("pre2_spill", [128, 2, 4, NP], BF16)
    cc_in = [[nc.dram_tensor(f"cc{l}{b}_in", [128, 4, 2], F32) for b in range(2)]
             for l in range(2)]
    cc_out = [[nc.dram_tensor(f"cc{l}{b}_out", [NCORES, 128, 4, 2], F32,
                              addr_space="Shared") for b in range(2)]
              for l in range(2)]

    with tile.TileContext(nc) as tc:
        with (
            tc.tile_pool(name="wpool", bufs=1) as wp,
            tc.tile_pool(name="stat", bufs=1) as stat,
        ):
            # ---- constants via cheap DVE memsets -------------------------
            ones_bf = wp.tile([128, 4], BF16, name="ones_bf")
            nc.vector.memset(ones_bf[:], 1.0)
            ones_r = wp.tile([128, 4], F32R, name="ones_r")
            nc.vector.memset(ones_r[:], 1.0)
            eps_t = stat.tile([128, 1], F32, name="eps_t")
            nc.vector.memset(eps_t[:], EPS)

            # W1 now (pass-1 projection needs it); the rest deferred.
            w1_bf, w1_r = [], []
            for b in range(2):
                w1b = wp.tile([128, 512], BF16, name=f"w1b_{b}")
                nc.scalar.dma_start(w1b[:], w1_d[b][:, :])
                w1r = wp.tile([128, 512], F32R, name=f"w1r_{b}")
                nc.vector.tensor_copy(w1r[:], w1b[:])
                w1_bf.append(w1b)
                w1_r.append(w1r)

            w2_t = [wp.tile([128, 4, 512], F32R, name=f"w2_{b}") for b in range(2)]
            m_t = [wp.tile([128, 4, 512], BF16, name=f"m_{b}") for b in range(2)]
            wh2_t = wp.tile([128, 4, 10], BF16, name="wh2_t")
            bp_sb = wp.tile([128, 4], F32, name="bp_sb")
            bh2_sb = wp.tile([10, 1], F32, name="bh2_sb")
            g_sb = [stat.tile([128, 2, 4], F32, name=f"g_sb{l}") for l in range(2)]
            be_sb = [stat.tile([128, 2, 4], F32, name=f"be_sb{l}") for l in range(2)]

            def load_pass2_weights():
                for b in range(2):
                    nc.scalar.dma_start(w2_t[b][:], w2_d[b][:, :, :])
                for l in range(2):
                    for b in range(2):
                        nc.scalar.dma_start(g_sb[l][:, b, :], g_d[l][b][:, :])
                        nc.scalar.dma_start(be_sb[l][:, b, :], be_d[l][b][:, :])

            def load_pass3_weights():
                for b in range(2):
                    nc.scalar.dma_start(m_t[b][:], m_d[b][:, :, :])
                nc.scalar.dma_start(wh2_t[:], wh2_d[:, :, :])
                nc.scalar.dma_start(bp_sb[:], bp_d[:, :])
                nc.scalar.dma_start(bh2_sb[:], bh2_d[:, :])

            st2 = stat.tile([128, 2, 4, C, 6], F32, name="st2")
            pay = [[stat.tile([128, 4, 2], F32, name=f"pay{l}{b}")
                    for b in range(2)] for l in range(2)]
            scale_t = [stat.tile([128, 2, 4], F32, name=f"scale{l}") for l in range(2)]
            shift_t = [stat.tile([128, 2, 4], F32, name=f"shift{l}") for l in range(2)]

            def issue_allreduce(l, b):
                # payload DMA on the Pool queue: SP/ACT SEQs must not
                # head-of-line block behind its wait for the stats payload
                nc.gpsimd.dma_start(cc_in[l][b][:, :, :], pay[l][b][:])
                # AllGather + local sum: the collective cost model charges
                # AllReduce 1.875x the gather time for the same tiny payload
                nc.gpsimd.collective_compute(
                    "AllGather", mybir.AluOpType.bypass,
                    replica_groups=[list(range(NCORES))],
                    ins=[cc_in[l][b].ap().opt()], outs=[cc_out[l][b].ap().opt()],
                )

            def finish_stats(l, b):
                """cc_out[l][b] -> scale_t[l][:, b, :], shift_t[l][:, b, :]."""
                gl8 = stat.tile([128, NCORES, 4, 2], F32, tag="gl8",
                                name=f"gl8{l}{b}")
                nc.gpsimd.dma_start(
                    gl8[:], cc_out[l][b].ap().rearrange("n p m s -> p n m s"))
                gl = stat.tile([128, 4, 2], F32, tag="gl", name=f"gl{l}{b}")
                nc.vector.tensor_add(gl[:], gl8[:, 0, :, :], gl8[:, 1, :, :])
                for n in range(2, NCORES):
                    nc.vector.tensor_add(gl[:], gl[:], gl8[:, n, :, :])
                mu = stat.tile([128, 4], F32, tag="mu", name=f"mu{l}{b}")
                var = stat.tile([128, 4], F32, tag="var", name=f"var{l}{b}")
                tmp = stat.tile([128, 4], F32, tag="tmpf", name=f"tmp{l}{b}")
                nc.vector.tensor_scalar_mul(mu[:], gl[:, :, 0], 1.0 / NTOT)
                nc.vector.tensor_scalar_mul(var[:], gl[:, :, 1], 1.0 / NTOT)
                nc.vector.tensor_mul(tmp[:], mu[:], mu[:])
                nc.vector.tensor_sub(var[:], var[:], tmp[:])
                nc.scalar.activation(var[:], var[:], AF.Sqrt, bias=eps_t[:])
                nc.vector.reciprocal(var[:], var[:])
                nc.vector.tensor_mul(scale_t[l][:, b, :], g_sb[l][:, b, :], var[:])
                nc.vector.tensor_mul(tmp[:], mu[:], scale_t[l][:, b, :])
                nc.vector.tensor_sub(shift_t[l][:, b, :], be_sb[l][:, b, :], tmp[:])

            # ================= pass 1: Gram + xsum -> BN1 stats ==========
            # explicit pool lifetimes: pass-3 PSUM (ps_t/ps_o) must reuse the
            # banks of the PASS-1 pools (drained by ~30us), not pass-2's --
            # otherwise pass-3's prologue matmuls serialize behind the whole
            # of pass 2 on the pool-drain barrier.
            ps_p1 = tc.alloc_tile_pool(name="ps_p1", bufs=2, space="PSUM")
            ps_p2 = tc.alloc_tile_pool(name="ps_p2", bufs=2, space="PSUM")
            xtp0 = tc.alloc_tile_pool(name="xtp0", bufs=1)
            parkp = tc.alloc_tile_pool(name="parkp", bufs=1)
            w2p = tc.alloc_tile_pool(name="w2p", bufs=2)
            g1p = tc.alloc_tile_pool(name="g1p", bufs=1)
            ps_g = tc.alloc_tile_pool(name="ps_g", bufs=1, space="PSUM")
            ps_xs = tc.alloc_tile_pool(name="ps_xs", bufs=1, space="PSUM")
            ps_pj = tc.alloc_tile_pool(name="ps_pj", bufs=1, space="PSUM")
            if True:
                def gram_pass(b, group_hook=None):
                    g_ps = ps_g.tile([128, 128], F32, tag="G", name=f"G_{b}")
                    xs_ps = ps_xs.tile([128, 4], F32, tag="XS", name=f"XS_{b}")
                    si = 0
                    for gi, (j0, gsz) in enumerate(GRAM_GROUPS):
                        xgt = g1p.tile([128, GGRP, 128], BF16, tag="xg",
                                       bufs=3, name=f"xg_{b}_{j0}")
                        nc.sync.dma_start(xgt[:, :gsz, :],
                                          xg_d[b][:, j0:j0 + gsz, :])
                        for j in range(gsz):
                            nc.tensor.matmul(g_ps[:], xgt[:, j, :], xgt[:, j, :],
                                             start=(si == 0), stop=(si == NSUB - 1))
                            nc.tensor.matmul(xs_ps[:], xgt[:, j, :], ones_bf[:],
                                             start=(si == 0), stop=(si == NSUB - 1))
                            si += 1
                        if group_hook:
                            group_hook(gi)
                    return g_ps, xs_ps

                def proj_pass(b, g_ps, xs_ps):
                    g_sbuf = g1p.tile([128, 128], F32R, tag="gsb", name=f"gsb_{b}")
                    nc.vector.tensor_copy(g_sbuf[:], g_ps[:])
                    xsum_r = g1p.tile([128, 4], F32R, tag="xsumr", name=f"xsumr_{b}")
                    nc.vector.tensor_copy(xsum_r[:], xs_ps[:])
                    mm1 = ps_pj.tile([128, 512], F32, tag="pj", name=f"mm1_{b}")
                    nc.tensor.matmul(mm1[:], g_sbuf[:], w1_r[b][:], start=True,
                                     stop=True)
                    mm1_sb = g1p.tile([128, 512], F32R, tag="mm1sb",
                                      name=f"mm1sb_{b}")
                    nc.vector.tensor_copy(mm1_sb[:], mm1[:])
                    prod = g1p.tile([128, 512], F32R, tag="prod", name=f"prod_{b}")
                    nc.vector.tensor_mul(prod[:], w1_r[b][:], mm1_sb[:])
                    for m in range(4):
                        sq = ps_pj.tile([128, 4], F32, tag="pj2", name=f"sq_{b}_{m}")
                        nc.tensor.matmul(sq[:], prod[:, ts(m, 128)], ones_r[:],
                                         start=True, stop=True)
                        nc.vector.tensor_copy(pay[0][b][:, m, 1:2], sq[:, 0:1])
                        sm = ps_pj.tile([128, 4], F32, tag="pj2", name=f"sm_{b}_{m}")
                        nc.tensor.matmul(sm[:], w1_r[b][:, ts(m, 128)], xsum_r[:],
                                         start=True, stop=True)
                        nc.vector.tensor_copy(pay[0][b][:, m, 0:1], sm[:, 0:1])
                    issue_allreduce(0, b)

                # pass-2 b0 input loads are emitted interleaved with the gram
                # loads so the PE gets stats-independent L1 work from ~3us
                xt0_tiles = {}

                def emit_xt0(c):
                    if c >= C or c in xt0_tiles:
                        return
                    c0, tc_sz = CHUNKS[c]
                    xt = xtp0.tile([128, tc_sz], BF16, tag=f"xt0_{c}",
                                   name=f"xt0_{c}")
                    nc.sync.dma_start(xt[:], xT_d[0][:, c0:c0 + tc_sz])
                    xt0_tiles[c] = xt

                g0, xs0 = gram_pass(0)
                proj_pass(0, g0, xs0)
                for c in range(C):
                    emit_xt0(c)
                g1, xs1 = gram_pass(1)
                proj_pass(1, g1, xs1)
                load_pass2_weights()

                # pass-1 pools drain by ~30us; their PSUM banks become the
                # pass-3 accumulation banks (no dependency on pass-2 pools)
                ps_pj.release()
                ps_xs.release()
                ps_g.release()
                g1p.release()
                ps_t = tc.alloc_tile_pool(name="ps_t", bufs=3, space="PSUM",
                                          side="right")
                ps_o = tc.alloc_tile_pool(name="ps_o", bufs=1, space="PSUM",
                                          side="right")

                # ---- pass-2 b0 loop A: L1 + park (stats-independent) ----
                PARK_C = 16  # chunks parked ahead of the first AllReduce
                park_tiles = {}
                for c, (c0, tc_sz) in enumerate(CHUNKS[:PARK_C]):
                    park = parkp.tile([128, 4, tc_sz], BF16, tag=f"park_{c}",
                                      name=f"park_{c}")
                    park_tiles[c] = park
                    for m in range(4):
                        pp = ps_p1.tile([128, tc_sz], F32, tag="p1",
                                        name=f"p1a_{c}_{m}")
                        nc.tensor.matmul(pp[:], w1_bf[0][:, ts(m, 128)],
                                         xt0_tiles[c][:], start=True, stop=True)
                        if m % 2 == 0:
                            nc.scalar.copy(park[:, m, :], pp[:])
                        else:
                            nc.vector.tensor_copy(park[:, m, :], pp[:])

                # ---- pass-2 b0 loop B: BN1 -> L2 -> stats/spill ---------
                finish_stats(0, 0)
                finish_stats(0, 1)
                for c, (c0, tc_sz) in enumerate(CHUNKS):
                    h1 = w2p.tile([128, 4, tc_sz], F32R, tag="h1",
                                  name=f"h1_{c}_0")
                    if c < PARK_C:
                        park = park_tiles.pop(c)
                        for m in range(4):
                            nc.scalar.activation(
                                h1[:, m, :], park[:, m, :], AF.Relu,
                                bias=shift_t[0][:, 0, m:m + 1],
                                scale=scale_t[0][:, 0, m:m + 1])
                    else:
                        for m in range(4):
                            pp = ps_p1.tile([128, tc_sz], F32, tag="p1",
                                            name=f"p1a_{c}_{m}")
                            nc.tensor.matmul(pp[:], w1_bf[0][:, ts(m, 128)],
                                             xt0_tiles[c][:], start=True,
                                             stop=True)
                            nc.scalar.activation(
                                h1[:, m, :], pp[:], AF.Relu,
                                bias=shift_t[0][:, 0, m:m + 1],
                                scale=scale_t[0][:, 0, m:m + 1])
                    if c == C - 1:
                        nc.scalar.mul(h1[:, :, PAD0:], h1[:, :, PAD0:], 0.0)
                    spl = w2p.tile([128, 4, tc_sz], BF16, tag="spl",
                                   name=f"spl_{c}_0", bufs=4)
                    for m in range(4):
                        pq = ps_p2.tile([128, tc_sz], F32, tag="p2",
                                        name=f"p2_{c}_0_{m}")
                        for k in range(4):
                            nc.tensor.matmul(pq[:], w2_t[0][:, k, ts(m, 128)],
                                             h1[:, k, :],
                                             start=(k == 0), stop=(k == 3))
                        if m % 2 == 0:
                            nc.scalar.copy(spl[:, m, :], pq[:])
                        else:
                            nc.vector.tensor_copy(spl[:, m, :], pq[:])
                    for m in range(4):
                        nc.vector.bn_stats(st2[:, 0, m, c, :], spl[:, m, :])
                    nc.sync.dma_start(spill[:, 0, :, c0:c0 + tc_sz], spl[:])

                agg = stat.tile([128, 4, 2], F32, tag="agg", name="agg_0")
                for m in range(4):
                    nc.vector.bn_aggr(agg[:, m, :], st2[:, 0, m, :, :])
                tmp2 = stat.tile([128, 4], F32, tag="tmp2", name="tmp2_0")
                nc.vector.tensor_scalar_mul(pay[1][0][:, :, 0], agg[:, :, 0],
                                            float(NP))
                nc.vector.tensor_mul(tmp2[:], agg[:, :, 0], agg[:, :, 0])
                nc.vector.tensor_add(tmp2[:], tmp2[:], agg[:, :, 1])
                nc.vector.tensor_scalar_mul(pay[1][0][:, :, 1], tmp2[:],
                                            float(NP))
                issue_allreduce(1, 0)
                finish_stats(1, 0)

                pf3_tiles = {}
                for c in range(3):
                    c0, tc_sz = CHUNKS[c]
                    pf = stat.tile([128, 4, tc_sz], BF16, tag=f"pf3_{c}",
                                   name=f"pf3_{c}")
                    nc.sync.dma_start(pf[:], spill[:, 0, :, c0:c0 + tc_sz])
                    pf3_tiles[c] = pf

                # ---- pass-2 b1 (direct PSUM path; stats(0,1) long ready) -
                load_pass3_weights()
                for c, (c0, tc_sz) in enumerate(CHUNKS):
                    xt = xtp0.tile([128, tc_sz], BF16, tag="xt1",
                                   name=f"xt1_{c}", bufs=8)
                    nc.sync.dma_start(xt[:], xT_d[1][:, c0:c0 + tc_sz])
                    h1 = w2p.tile([128, 4, tc_sz], F32R, tag="h1",
                                  name=f"h1_{c}_1")
                    for m in range(4):
                        pp = ps_p1.tile([128, tc_sz], F32, tag="p1",
                                        name=f"p1b_{c}_{m}")
                        nc.tensor.matmul(pp[:], w1_bf[1][:, ts(m, 128)],
                                         xt[:], start=True, stop=True)
                        nc.scalar.activation(
                            h1[:, m, :], pp[:], AF.Relu,
                            bias=shift_t[0][:, 1, m:m + 1],
                            scale=scale_t[0][:, 1, m:m + 1])
                    if c == C - 1:
                        nc.scalar.mul(h1[:, :, PAD0:], h1[:, :, PAD0:], 0.0)
                    spl = w2p.tile([128, 4, tc_sz], BF16, tag="spl",
                                   name=f"spl_{c}_1", bufs=4)
                    for m in range(4):
                        pq = ps_p2.tile([128, tc_sz], F32, tag="p2",
                                        name=f"p2_{c}_1_{m}")
                        for k in range(4):
                            nc.tensor.matmul(pq[:], w2_t[1][:, k, ts(m, 128)],
                                             h1[:, k, :],
                                             start=(k == 0), stop=(k == 3))
                        if m % 2 == 0 and c < 20:
                            nc.scalar.copy(spl[:, m, :], pq[:])
                        else:
                            nc.vector.tensor_copy(spl[:, m, :], pq[:])
                    for m in range(4):
                        nc.vector.bn_stats(st2[:, 1, m, c, :], spl[:, m, :])
                    nc.sync.dma_start(spill[:, 1, :, c0:c0 + tc_sz], spl[:])

                agg1 = stat.tile([128, 4, 2], F32, tag="agg", name="agg_1")
                for m in range(4):
                    nc.vector.bn_aggr(agg1[:, m, :], st2[:, 1, m, :, :])
                tmp21 = stat.tile([128, 4], F32, tag="tmp2", name="tmp2_1")
                nc.vector.tensor_scalar_mul(pay[1][1][:, :, 0], agg1[:, :, 0],
                                            float(NP))
                nc.vector.tensor_mul(tmp21[:], agg1[:, :, 0], agg1[:, :, 0])
                nc.vector.tensor_add(tmp21[:], tmp21[:], agg1[:, :, 1])
                nc.vector.tensor_scalar_mul(pay[1][1][:, :, 1], tmp21[:],
                                            float(NP))
                issue_allreduce(1, 1)

            w2p.release()
            parkp.release()
            xtp0.release()
            ps_p2.release()
            ps_p1.release()

            # ================= pass 3: BN2 -> fused M -> head ============
            PRO3 = 6  # chunks whose b0-half runs ahead of the last AllReduce
            with (
                tc.tile_pool(name="w3p", bufs=2) as w3p,
                tc.tile_pool(name="tpark", bufs=1) as tpark,
            ):
                def load_pre2(c, b):
                    c0, tc_sz = CHUNKS[c]
                    t = w3p.tile([128, 4, tc_sz], BF16, tag=f"pre2_{b}",
                                 bufs=3, name=f"pre2_{c}_{b}")
                    nc.sync.dma_start(t[:], spill[:, b, :, c0:c0 + tc_sz])
                    return t

                def h2_act(c, b, pre2):
                    """branch-0 on ACT (1 op/slab); branch-1 on DVE (2 ops)."""
                    c0, tc_sz = CHUNKS[c]
                    h2 = w3p.tile([128, 4, tc_sz], BF16, tag=f"h2_{b}",
                                  bufs=3, name=f"h2_{c}_{b}")
                    for k in range(4):
                        if b == 0:
                            nc.scalar.activation(
                                h2[:, k, :], pre2[:, k, :], AF.Relu,
                                bias=shift_t[1][:, 0, k:k + 1],
                                scale=scale_t[1][:, 0, k:k + 1])
                        else:
                            nc.vector.tensor_scalar(
                                h2[:, k, :], pre2[:, k, :],
                                scale_t[1][:, 1, k:k + 1],
                                shift_t[1][:, 1, k:k + 1],
                                ALU.mult, ALU.add)
                            nc.vector.tensor_scalar_max(h2[:, k, :],
                                                        h2[:, k, :], 0.0)
                    return h2

                def head(c, t_sb):
                    c0, tc_sz = CHUNKS[c]
                    po = ps_o.tile([10, tc_sz], F32, tag="o", name=f"po_{c}")
                    for k in range(4):
                        nc.tensor.matmul(po[:], wh2_t[:, k, :], t_sb[:, k, :],
                                         start=(k == 0), stop=(k == 3))
                    o_sb = w3p.tile([10, tc_sz], F32, tag="o_sb", name=f"o_sb_{c}")
                    nc.scalar.activation(o_sb[:], po[:], AF.Identity,
                                         bias=bh2_sb[:, 0:1])
                    nc.sync.dma_start(outd[:, c0:c0 + tc_sz], o_sb[:])

                # prologue: b0-halves of the first PRO3 chunks run while
                # AllReduce(1,1) is still in flight; partials parked as bf16
                parked_t = {}
                for c in range(PRO3):
                    c0, tc_sz = CHUNKS[c]
                    pre2_0 = pf3_tiles.pop(c) if c in pf3_tiles else \
                        load_pre2(c, 0)
                    h2_0 = h2_act(c, 0, pre2_0)
                    tp = tpark.tile([128, 4, tc_sz], BF16, tag=f"tp_{c}",
                                    name=f"tp_{c}")
                    parked_t[c] = tp
                    for m in range(4):
                        ptl = ps_t.tile([128, tc_sz], F32, tag="t",
                                        name=f"ptl_{c}_{m}a")
                        for k in range(4):
                            nc.tensor.matmul(ptl[:], m_t[0][:, k, ts(m, 128)],
                                             h2_0[:, k, :],
                                             start=(k == 0), stop=(k == 3))
                        if m % 2 == 0:
                            nc.scalar.copy(tp[:, m, :], ptl[:])
                        else:
                            nc.vector.tensor_copy(tp[:, m, :], ptl[:])

                finish_stats(1, 1)

                # combine stage for prologue chunks (needs stats(1,1))
                for c in range(PRO3):
                    c0, tc_sz = CHUNKS[c]
                    h2_1 = h2_act(c, 1, load_pre2(c, 1))
                    t_sb = w3p.tile([128, 4, tc_sz], BF16, tag="t_sb", bufs=2,
                                    name=f"t_sb_{c}")
                    tp = parked_t.pop(c)
                    for m in range(4):
                        ptl = ps_t.tile([128, tc_sz], F32, tag="t",
                                        name=f"ptl_{c}_{m}b")
                        for k in range(4):
                            nc.tensor.matmul(ptl[:], m_t[1][:, k, ts(m, 128)],
                                             h2_1[:, k, :],
                                             start=(k == 0), stop=(k == 3))
                        # t_pre = (psum + bias) + parked, then relu
                        nc.vector.scalar_tensor_tensor(
                            t_sb[:, m, :], ptl[:], bp_sb[:, m:m + 1],
                            tp[:, m, :], ALU.add, ALU.add)
                    t_sb2 = w3p.tile([128, 4, tc_sz], BF16, tag="t_sb2", bufs=2,
                                     name=f"t_sb2_{c}")
                    nc.scalar.activation(t_sb2[:, :, :], t_sb[:, :, :], AF.Relu)
                    head(c, t_sb2)

                # steady state: full chunk in one pass
                for c in range(PRO3, C):
                    c0, tc_sz = CHUNKS[c]
                    h2_0 = h2_act(c, 0, load_pre2(c, 0))
                    h2_1 = h2_act(c, 1, load_pre2(c, 1))
                    t_sb = w3p.tile([128, 4, tc_sz], BF16, tag="t_sb", bufs=2,
                                    name=f"t_sb_{c}")
                    for m in range(4):
                        ptl = ps_t.tile([128, tc_sz], F32, tag="t",
                                        name=f"ptl_{c}_{m}")
                        for k in range(4):
                            nc.tensor.matmul(ptl[:], m_t[0][:, k, ts(m, 128)],
                                             h2_0[:, k, :],
                                             start=(k == 0), stop=False)
                        for k in range(4):
                            nc.tensor.matmul(ptl[:], m_t[1][:, k, ts(m, 128)],
                                             h2_1[:, k, :],
                                             start=False, stop=(k == 3))
                        nc.scalar.activation(t_sb[:, m, :], ptl[:], AF.Relu,
                                             bias=bp_sb[:, m:m + 1])
                    head(c, t_sb)

            ps_o.release()
            ps_t.release()

    nc.compile()
    return nc


def _get_program():
    if "nc" not in _CACHE:
        _CACHE["nc"] = _build_program()
    return _CACHE["nc"]


def kernel(**inputs):
    import ml_dtypes

    nc = _get_program()
    bf16 = ml_dtypes.bfloat16

    def shard_x(x):
        x = np.ascontiguousarray(x, dtype=np.float32).reshape(NCORES, NSH, 128)
        pad = np.zeros((NCORES, NP - NSH, 128), dtype=np.float32)
        return np.concatenate([x, pad], axis=1)  # [NCORES, NP, 128]

    xs = [shard_x(inputs["x_1"]), shard_x(inputs["x_2"])]
    # feature-major + gram layouts, bf16
    xT = [np.ascontiguousarray(x.transpose(0, 2, 1)).astype(bf16) for x in xs]
    xg = [np.ascontiguousarray(x.reshape(NCORES, 128, NSUB, 128)).astype(bf16)
          for x in xs]

    def km(w):  # [512, O] -> [128, 4, O] (contraction-major for lhsT slabs)
        O = w.shape[1]
        return np.ascontiguousarray(
            w.reshape(4, 128, O).transpose(1, 0, 2))

    def vec(v):  # [512] -> [128, 4]
        return np.ascontiguousarray(v.reshape(4, 128).T)

    f64 = np.float64
    Wh1 = np.asarray(inputs["Wh1"], f64)
    rep = {}
    for b, sfx in ((0, "1"), (1, "2")):
        rep[f"W1_{b}"] = np.asarray(inputs[f"W1_{sfx}"], np.float32).astype(bf16)
        rep[f"W2_{b}"] = km(np.asarray(inputs[f"W2_{sfx}"], np.float32))
        M = np.asarray(inputs[f"Wf_{sfx}"], f64) @ Wh1[b * 512:(b + 1) * 512, :]
        rep[f"M_{b}"] = km(M.astype(np.float32)).astype(bf16)
        for l, nm in ((0, "1"), (1, "2")):
            rep[f"g{l}_{b}"] = vec(np.asarray(inputs[f"g{nm}_{sfx}"], np.float32))
            rep[f"be{l}_{b}"] = vec(np.asarray(inputs[f"be{nm}_{sfx}"], np.float32))
    bp = (np.asarray(inputs["bf_1"], f64) @ Wh1[:512, :]
          + np.asarray(inputs["bf_2"], f64) @ Wh1[512:, :]
          + np.asarray(inputs["bh1"], f64))
    rep["BP"] = vec(bp.astype(np.float32))
    rep["WH2"] = km(np.asarray(inputs["Wh2"], np.float32)).astype(bf16)
    rep["BH2"] = np.ascontiguousarray(
        np.asarray(inputs["bh2"], np.float32).reshape(10, 1))

    in_maps = []
    for c in range(NCORES):
        m = {"xT_0": xT[0][c], "xT_1": xT[1][c],
             "xg_0": xg[0][c], "xg_1": xg[1][c]}
        m.update(rep)
        in_maps.append(m)

    res = bass_utils.run_bass_kernel_spmd(nc, in_maps, core_ids=list(range(NCORES)))
    parts = [res.results[c]["OUT"][:, :NSH] for c in range(NCORES)]
    out = np.concatenate(parts, axis=1).T
    return np.ascontiguousarray(out, dtype=np.float32)


# revision 23
# speedup vs baseline: 1.4371x; 1.0096x over previous
"""ChebyNet (K=1) dual-branch MLP + BN kernel for 8 Trainium2 NeuronCores.

Network (per reference):
  branch b in {1,2}:  h = relu(BN(x_b @ W1_b)) ; h = relu(BN(h @ W2_b)) ; f_b = h @ Wf_b + bf_b
  out = relu(concat(f_1, f_2) @ Wh1 + bh1) @ Wh2 + bh2

ChebConv with K=1 ignores edge_index/edge_weight entirely.  Training-mode
BatchNorm over the node axis makes the linear-layer biases b1/b2 cancel
exactly, so they are never loaded.

Key restructurings vs the direct form:
  * Wf_b and Wh1 compose linearly (no nonlinearity between them), so the
    host folds M_b = Wf_b @ Wh1[b-half] and b' = bf_1 @ Wh1a + bf_2 @ Wh1b
    + bh1.  The Lf layer and the concat disappear: t = relu(h2_1 @ M_1 +
    h2_2 @ M_2 + b'), out = t @ Wh2 + bh2.
  * The host pre-transposes x into feature-major xT (bf16) and a
    partition-major node layout xg for the Gram pass, so the kernel never
    runs PE transposes.
  * Layer-1 BN stats use the Gram identity: sumsq(pre1) = diag(W1^T (X^T X)
    W1), sum(pre1) = W1^T (X^T 1).  X^T 1 comes from the same Gram
    stationary tiles with a [128,4] ones moving operand (nearly free).
  * Pass 2 computes branch-0 L1 for all chunks before BN1 stats arrive,
    parking pre1 as bf16 ("loop A"); the BN1+relu+L2 pass ("loop B") then
    never stalls on the first AllReduce.

Sharding: nodes (axis 0) split across 8 cores, 12500 each, zero-padded to
12544 = 98*128.  Weights replicated.  BN batch stats are combined with an
AllReduce(add) of per-core (sum, sumsq); the four collectives are
interleaved so each hides under the next phase's compute.
"""

import os

os.environ.setdefault("JAX_PLATFORMS", "axon,cpu")

import numpy as np

import concourse.bacc as bacc
import concourse.mybir as mybir
import concourse.tile as tile
from concourse import bass_utils
from concourse.bass import ts

F32 = mybir.dt.float32
F32R = mybir.dt.float32r
BF16 = mybir.dt.bfloat16
AF = mybir.ActivationFunctionType
ALU = mybir.AluOpType

NTOT = 100000          # true node count
NCORES = 8
NSH = NTOT // NCORES   # 12500 true nodes per core
NP = 12544             # padded per-core nodes (= 98 * 128)
T = 512                # node-chunk size (free dim of matmuls / PSUM bank)
CHUNKS = [(i * T, T) for i in range(NP // T)] + ([(NP - NP % T, NP % T)] if NP % T else [])
C = len(CHUNKS)
NSUB = NP // 128       # 98 gram sub-tiles
GGRP = 49              # gram sub-tiles per DMA (2 loads per branch)
GRAM_GROUPS = [(0, GGRP), (GGRP, GGRP)]
XSLAB = [(0, 3072), (3072, 3072), (6144, 3072), (9216, NP - 9216)]
PAD0 = NSH - (NP - (NP % T or T))  # first padded column inside last chunk (212)
EPS = 1e-5

_CACHE = {}


def _build_program():
    nc = bacc.Bacc("TRN2", target_bir_lowering=False, debug=False,
                   num_devices=NCORES)

    # ---- kernel I/O -----------------------------------------------------
    xT_d = [nc.dram_tensor(f"xT_{b}", [128, NP], BF16, kind="ExternalInput")
            for b in range(2)]
    xg_d = [nc.dram_tensor(f"xg_{b}", [128, NSUB, 128], BF16,
                           kind="ExternalInput") for b in range(2)]
    w1_d = [nc.dram_tensor(f"W1_{b}", [128, 512], BF16, kind="ExternalInput")
            for b in range(2)]
    w2_d = [nc.dram_tensor(f"W2_{b}", [128, 4, 512], F32R,
                           kind="ExternalInput") for b in range(2)]
    m_d = [nc.dram_tensor(f"M_{b}", [128, 4, 512], BF16, kind="ExternalInput")
           for b in range(2)]
    wh2_d = nc.dram_tensor("WH2", [128, 4, 10], BF16, kind="ExternalInput")
    bp_d = nc.dram_tensor("BP", [128, 4], F32, kind="ExternalInput")
    bh2_d = nc.dram_tensor("BH2", [10, 1], F32, kind="ExternalInput")
    g_d = [[nc.dram_tensor(f"g{l}_{b}", [128, 4], F32, kind="ExternalInput")
            for b in range(2)] for l in range(2)]
    be_d = [[nc.dram_tensor(f"be{l}_{b}", [128, 4], F32, kind="ExternalInput")
             for b in range(2)] for l in range(2)]
    outd = nc.dram_tensor("OUT", [10, NP], F32, kind="ExternalOutput")

    # ---- DRAM scratch ---------------------------------------------------
    spill = nc.dram_tensor("pre2_spill", [128, 2, 4, NP], BF16)
    cc_in = [[nc.dram_tensor(f"cc{l}{b}_in", [128, 4, 2], F32) for b in range(2)]
             for l in range(2)]
    cc_out = [[nc.dram_tensor(f"cc{l}{b}_out", [NCORES, 128, 4, 2], F32,
                              addr_space="Shared") for b in range(2)]
              for l in range(2)]

    with tile.TileContext(nc) as tc:
        with (
            tc.tile_pool(name="wpool", bufs=1) as wp,
            tc.tile_pool(name="stat", bufs=1) as stat,
        ):
            # ---- constants via cheap DVE memsets -------------------------
            ones_bf = wp.tile([128, 4], BF16, name="ones_bf")
            nc.vector.memset(ones_bf[:], 1.0)
            ones_r = wp.tile([128, 4], F32R, name="ones_r")
            nc.vector.memset(ones_r[:], 1.0)
            eps_t = stat.tile([128, 1], F32, name="eps_t")
            nc.vector.memset(eps_t[:], EPS)

            # W1 now (pass-1 projection needs it); the rest deferred.
            w1_bf, w1_r = [], []
            for b in range(2):
                w1b = wp.tile([128, 512], BF16, name=f"w1b_{b}")
                nc.scalar.dma_start(w1b[:], w1_d[b][:, :])
                w1r = wp.tile([128, 512], F32R, name=f"w1r_{b}")
                nc.vector.tensor_copy(w1r[:], w1b[:])
                w1_bf.append(w1b)
                w1_r.append(w1r)

            w2_t = [wp.tile([128, 4, 512], F32R, name=f"w2_{b}") for b in range(2)]
            m_t = [wp.tile([128, 4, 512], BF16, name=f"m_{b}") for b in range(2)]
            wh2_t = wp.tile([128, 4, 10], BF16, name="wh2_t")
            bp_sb = wp.tile([128, 4], F32, name="bp_sb")
            bh2_sb = wp.tile([10, 1], F32, name="bh2_sb")
            g_sb = [stat.tile([128, 2, 4], F32, name=f"g_sb{l}") for l in range(2)]
            be_sb = [stat.tile([128, 2, 4], F32, name=f"be_sb{l}") for l in range(2)]

            def load_pass2_weights():
                for b in range(2):
                    nc.scalar.dma_start(w2_t[b][:], w2_d[b][:, :, :])
                for l in range(2):
                    for b in range(2):
                        nc.scalar.dma_start(g_sb[l][:, b, :], g_d[l][b][:, :])
                        nc.scalar.dma_start(be_sb[l][:, b, :], be_d[l][b][:, :])

            def load_pass3_weights():
                for b in range(2):
                    nc.scalar.dma_start(m_t[b][:], m_d[b][:, :, :])
                nc.scalar.dma_start(wh2_t[:], wh2_d[:, :, :])
                nc.scalar.dma_start(bp_sb[:], bp_d[:, :])
                nc.scalar.dma_start(bh2_sb[:], bh2_d[:, :])

            st2 = stat.tile([128, 2, 4, C, 6], F32, name="st2")
            pay = [[stat.tile([128, 4, 2], F32, name=f"pay{l}{b}")
                    for b in range(2)] for l in range(2)]
            scale_t = [stat.tile([128, 2, 4], F32, name=f"scale{l}") for l in range(2)]
            shift_t = [stat.tile([128, 2, 4], F32, name=f"shift{l}") for l in range(2)]

            def issue_allreduce(l, b):
                # payload DMA on the Pool queue: SP/ACT SEQs must not
                # head-of-line block behind its wait for the stats payload
                nc.gpsimd.dma_start(cc_in[l][b][:, :, :], pay[l][b][:])
                # AllGather + local sum: the collective cost model charges
                # AllReduce 1.875x the gather time for the same tiny payload
                nc.gpsimd.collective_compute(
                    "AllGather", mybir.AluOpType.bypass,
                    replica_groups=[list(range(NCORES))],
                    ins=[cc_in[l][b].ap().opt()], outs=[cc_out[l][b].ap().opt()],
                )

            def finish_stats(l, b):
                """cc_out[l][b] -> scale_t[l][:, b, :], shift_t[l][:, b, :]."""
                gl8 = stat.tile([128, NCORES, 4, 2], F32, tag="gl8",
                                name=f"gl8{l}{b}")
                nc.gpsimd.dma_start(
                    gl8[:], cc_out[l][b].ap().rearrange("n p m s -> p n m s"))
                gl = stat.tile([128, 4, 2], F32, tag="gl", name=f"gl{l}{b}")
                nc.vector.tensor_add(gl[:], gl8[:, 0, :, :], gl8[:, 1, :, :])
                for n in range(2, NCORES):
                    nc.vector.tensor_add(gl[:], gl[:], gl8[:, n, :, :])
                mu = stat.tile([128, 4], F32, tag="mu", name=f"mu{l}{b}")
                var = stat.tile([128, 4], F32, tag="var", name=f"var{l}{b}")
                tmp = stat.tile([128, 4], F32, tag="tmpf", name=f"tmp{l}{b}")
                nc.vector.tensor_scalar_mul(mu[:], gl[:, :, 0], 1.0 / NTOT)
                nc.vector.tensor_scalar_mul(var[:], gl[:, :, 1], 1.0 / NTOT)
                nc.vector.tensor_mul(tmp[:], mu[:], mu[:])
                nc.vector.tensor_sub(var[:], var[:], tmp[:])
                nc.scalar.activation(var[:], var[:], AF.Sqrt, bias=eps_t[:])
                nc.vector.reciprocal(var[:], var[:])
                nc.vector.tensor_mul(scale_t[l][:, b, :], g_sb[l][:, b, :], var[:])
                nc.vector.tensor_mul(tmp[:], mu[:], scale_t[l][:, b, :])
                nc.vector.tensor_sub(shift_t[l][:, b, :], be_sb[l][:, b, :], tmp[:])

            # ================= pass 1: Gram + xsum -> BN1 stats ==========
            # explicit pool lifetimes: pass-3 PSUM (ps_t/ps_o) must reuse the
            # banks of the PASS-1 pools (drained by ~30us), not pass-2's --
            # otherwise pass-3's prologue matmuls serialize behind the whole
            # of pass 2 on the pool-drain barrier.
            ps_p1 = tc.alloc_tile_pool(name="ps_p1", bufs=2, space="PSUM")
            ps_p2 = tc.alloc_tile_pool(name="ps_p2", bufs=2, space="PSUM")
            xtp0 = tc.alloc_tile_pool(name="xtp0", bufs=1)
            parkp = tc.alloc_tile_pool(name="parkp", bufs=1)
            w2p = tc.alloc_tile_pool(name="w2p", bufs=2)
            g1p = tc.alloc_tile_pool(name="g1p", bufs=1)
            ps_g = tc.alloc_tile_pool(name="ps_g", bufs=1, space="PSUM")
            ps_xs = tc.alloc_tile_pool(name="ps_xs", bufs=1, space="PSUM")
            ps_pj = tc.alloc_tile_pool(name="ps_pj", bufs=1, space="PSUM")
            if True:
                def gram_load(b, gi):
                    j0, gsz = GRAM_GROUPS[gi]
                    xgt = g1p.tile([128, GGRP, 128], BF16, tag="xg",
                                   bufs=2, name=f"xg_{b}_{j0}")
                    nc.sync.dma_start(xgt[:, :gsz, :],
                                      xg_d[b][:, j0:j0 + gsz, :])
                    return xgt

                def gram_mms(b, gi, xgt, g_ps, xs_ps):
                    j0, gsz = GRAM_GROUPS[gi]
                    for j in range(gsz):
                        si = j0 + j
                        nc.tensor.matmul(g_ps[:], xgt[:, j, :], xgt[:, j, :],
                                         start=(si == 0), stop=(si == NSUB - 1))
                        nc.tensor.matmul(xs_ps[:], xgt[:, j, :], ones_bf[:],
                                         start=(si == 0), stop=(si == NSUB - 1))

                def gram_psum(b):
                    g_ps = ps_g.tile([128, 128], F32, tag="G", name=f"G_{b}")
                    xs_ps = ps_xs.tile([128, 4], F32, tag="XS", name=f"XS_{b}")
                    return g_ps, xs_ps

                def proj_pass(b, g_ps, xs_ps):
                    g_sbuf = g1p.tile([128, 128], F32R, tag="gsb", name=f"gsb_{b}")
                    nc.vector.tensor_copy(g_sbuf[:], g_ps[:])
                    xsum_r = g1p.tile([128, 4], F32R, tag="xsumr", name=f"xsumr_{b}")
                    nc.vector.tensor_copy(xsum_r[:], xs_ps[:])
                    mm1 = ps_pj.tile([128, 512], F32, tag="pj", name=f"mm1_{b}")
                    nc.tensor.matmul(mm1[:], g_sbuf[:], w1_r[b][:], start=True,
                                     stop=True)
                    mm1_sb = g1p.tile([128, 512], F32R, tag="mm1sb",
                                      name=f"mm1sb_{b}")
                    nc.vector.tensor_copy(mm1_sb[:], mm1[:])
                    prod = g1p.tile([128, 512], F32R, tag="prod", name=f"prod_{b}")
                    nc.vector.tensor_mul(prod[:], w1_r[b][:], mm1_sb[:])
                    for m in range(4):
                        sq = ps_pj.tile([128, 4], F32, tag="pj2", name=f"sq_{b}_{m}")
                        nc.tensor.matmul(sq[:], prod[:, ts(m, 128)], ones_r[:],
                                         start=True, stop=True)
                        nc.vector.tensor_copy(pay[0][b][:, m, 1:2], sq[:, 0:1])
                        sm = ps_pj.tile([128, 4], F32, tag="pj2", name=f"sm_{b}_{m}")
                        nc.tensor.matmul(sm[:], w1_r[b][:, ts(m, 128)], xsum_r[:],
                                         start=True, stop=True)
                        nc.vector.tensor_copy(pay[0][b][:, m, 0:1], sm[:, 0:1])
                    issue_allreduce(0, b)

                # branch-0 input (xT) lives resident in 4 slab tiles whose
                # loads interleave with the gram loads: few big DMA
                # instructions (the SP->HWDGE issue rate is ~1.3us/instr)
                xt0_slabs = [xtp0.tile([128, sz], BF16, tag=f"xts_{i}",
                                       name=f"xts_{i}")
                             for i, (s0, sz) in enumerate(XSLAB)]

                def emit_xts(i):
                    s0, sz = XSLAB[i]
                    nc.sync.dma_start(xt0_slabs[i][:], xT_d[0][:, s0:s0 + sz])

                def xt0_ap(c):
                    c0, tc_sz = CHUNKS[c]
                    for i, (s0, sz) in enumerate(XSLAB):
                        if s0 <= c0 and c0 + tc_sz <= s0 + sz:
                            return xt0_slabs[i][:, c0 - s0:c0 - s0 + tc_sz]
                    raise AssertionError(c)

                g0, xs0 = gram_psum(0)
                xg00 = gram_load(0, 0)
                xg01 = gram_load(0, 1)
                gram_mms(0, 0, xg00, g0, xs0)
                gram_mms(0, 1, xg01, g0, xs0)
                proj_pass(0, g0, xs0)
                emit_xts(0)
                g1, xs1 = gram_psum(1)
                xg10 = gram_load(1, 0)
                emit_xts(1)
                xg11 = gram_load(1, 1)
                gram_mms(1, 0, xg10, g1, xs1)
                emit_xts(2)
                emit_xts(3)
                gram_mms(1, 1, xg11, g1, xs1)
                proj_pass(1, g1, xs1)
                load_pass2_weights()

                # pass-1 pools drain by ~30us; their PSUM banks become the
                # pass-3 accumulation banks (no dependency on pass-2 pools)
                ps_pj.release()
                ps_xs.release()
                ps_g.release()
                g1p.release()
                ps_t = tc.alloc_tile_pool(name="ps_t", bufs=3, space="PSUM",
                                          side="right")
                ps_o = tc.alloc_tile_pool(name="ps_o", bufs=1, space="PSUM",
                                          side="right")

                # ---- pass-2 b0 loop A: L1 + park (stats-independent) ----
                PARK_C = 12  # chunks parked ahead of the first AllReduce
                park_tiles = {}
                for c, (c0, tc_sz) in enumerate(CHUNKS[:PARK_C]):
                    park = parkp.tile([128, 4, tc_sz], BF16, tag=f"park_{c}",
                                      name=f"park_{c}")
                    park_tiles[c] = park
                    for m in range(4):
                        pp = ps_p1.tile([128, tc_sz], F32, tag="p1",
                                        name=f"p1a_{c}_{m}")
                        nc.tensor.matmul(pp[:], w1_bf[0][:, ts(m, 128)],
                                         xt0_ap(c), start=True, stop=True)
                        if m % 2 == 0:
                            nc.scalar.copy(park[:, m, :], pp[:])
                        else:
                            nc.vector.tensor_copy(park[:, m, :], pp[:])

                # ---- pass-2 b0 loop B: BN1 -> L2 -> stats/spill ---------
                finish_stats(0, 0)
                finish_stats(0, 1)
                for c, (c0, tc_sz) in enumerate(CHUNKS):
                    h1 = w2p.tile([128, 4, tc_sz], F32R, tag="h1",
                                  name=f"h1_{c}_0")
                    if c < PARK_C:
                        park = park_tiles.pop(c)
                        for m in range(4):
                            nc.scalar.activation(
                                h1[:, m, :], park[:, m, :], AF.Relu,
                                bias=shift_t[0][:, 0, m:m + 1],
                                scale=scale_t[0][:, 0, m:m + 1])
                    else:
                        for m in range(4):
                            pp = ps_p1.tile([128, tc_sz], F32, tag="p1",
                                            name=f"p1a_{c}_{m}")
                            nc.tensor.matmul(pp[:], w1_bf[0][:, ts(m, 128)],
                                             xt0_ap(c), start=True,
                                             stop=True)
                            nc.scalar.activation(
                                h1[:, m, :], pp[:], AF.Relu,
                                bias=shift_t[0][:, 0, m:m + 1],
                                scale=scale_t[0][:, 0, m:m + 1])
                    if c == C - 1:
                        nc.scalar.mul(h1[:, :, PAD0:], h1[:, :, PAD0:], 0.0)
                    spl = w2p.tile([128, 4, tc_sz], BF16, tag="spl",
                                   name=f"spl_{c}_0", bufs=4)
                    for m in range(4):
                        pq = ps_p2.tile([128, tc_sz], F32, tag="p2",
                                        name=f"p2_{c}_0_{m}")
                        for k in range(4):
                            nc.tensor.matmul(pq[:], w2_t[0][:, k, ts(m, 128)],
                                             h1[:, k, :],
                                             start=(k == 0), stop=(k == 3))
                        if m % 2 == 0:
                            nc.scalar.copy(spl[:, m, :], pq[:])
                        else:
                            nc.vector.tensor_copy(spl[:, m, :], pq[:])
                    for m in range(4):
                        nc.vector.bn_stats(st2[:, 0, m, c, :], spl[:, m, :])
                    nc.sync.dma_start(spill[:, 0, :, c0:c0 + tc_sz], spl[:])

                agg = stat.tile([128, 4, 2], F32, tag="agg", name="agg_0")
                for m in range(4):
                    nc.vector.bn_aggr(agg[:, m, :], st2[:, 0, m, :, :])
                tmp2 = stat.tile([128, 4], F32, tag="tmp2", name="tmp2_0")
                nc.vector.tensor_scalar_mul(pay[1][0][:, :, 0], agg[:, :, 0],
                                            float(NP))
                nc.vector.tensor_mul(tmp2[:], agg[:, :, 0], agg[:, :, 0])
                nc.vector.tensor_add(tmp2[:], tmp2[:], agg[:, :, 1])
                nc.vector.tensor_scalar_mul(pay[1][0][:, :, 1], tmp2[:],
                                            float(NP))
                issue_allreduce(1, 0)
                finish_stats(1, 0)

                pf3_tiles = {}
                for c in range(3):
                    c0, tc_sz = CHUNKS[c]
                    pf = stat.tile([128, 4, tc_sz], BF16, tag=f"pf3_{c}",
                                   name=f"pf3_{c}")
                    nc.sync.dma_start(pf[:], spill[:, 0, :, c0:c0 + tc_sz])
                    pf3_tiles[c] = pf

                # ---- pass-2 b1 (direct PSUM path; stats(0,1) long ready) -
                load_pass3_weights()
                for c, (c0, tc_sz) in enumerate(CHUNKS):
                    xt = xtp0.tile([128, tc_sz], BF16, tag="xt1",
                                   name=f"xt1_{c}", bufs=8)
                    nc.sync.dma_start(xt[:], xT_d[1][:, c0:c0 + tc_sz])
                    h1 = w2p.tile([128, 4, tc_sz], F32R, tag="h1",
                                  name=f"h1_{c}_1")
                    for m in range(4):
                        pp = ps_p1.tile([128, tc_sz], F32, tag="p1",
                                        name=f"p1b_{c}_{m}")
                        nc.tensor.matmul(pp[:], w1_bf[1][:, ts(m, 128)],
                                         xt[:], start=True, stop=True)
                        nc.scalar.activation(
                            h1[:, m, :], pp[:], AF.Relu,
                            bias=shift_t[0][:, 1, m:m + 1],
                            scale=scale_t[0][:, 1, m:m + 1])
                    if c == C - 1:
                        nc.scalar.mul(h1[:, :, PAD0:], h1[:, :, PAD0:], 0.0)
                    spl = w2p.tile([128, 4, tc_sz], BF16, tag="spl",
                                   name=f"spl_{c}_1", bufs=4)
                    for m in range(4):
                        pq = ps_p2.tile([128, tc_sz], F32, tag="p2",
                                        name=f"p2_{c}_1_{m}")
                        for k in range(4):
                            nc.tensor.matmul(pq[:], w2_t[1][:, k, ts(m, 128)],
                                             h1[:, k, :],
                                             start=(k == 0), stop=(k == 3))
                        if m % 2 == 0 and c < 20:
                            nc.scalar.copy(spl[:, m, :], pq[:])
                        else:
                            nc.vector.tensor_copy(spl[:, m, :], pq[:])
                    for m in range(4):
                        nc.vector.bn_stats(st2[:, 1, m, c, :], spl[:, m, :])
                    nc.sync.dma_start(spill[:, 1, :, c0:c0 + tc_sz], spl[:])

                agg1 = stat.tile([128, 4, 2], F32, tag="agg", name="agg_1")
                for m in range(4):
                    nc.vector.bn_aggr(agg1[:, m, :], st2[:, 1, m, :, :])
                tmp21 = stat.tile([128, 4], F32, tag="tmp2", name="tmp2_1")
                nc.vector.tensor_scalar_mul(pay[1][1][:, :, 0], agg1[:, :, 0],
                                            float(NP))
                nc.vector.tensor_mul(tmp21[:], agg1[:, :, 0], agg1[:, :, 0])
                nc.vector.tensor_add(tmp21[:], tmp21[:], agg1[:, :, 1])
                nc.vector.tensor_scalar_mul(pay[1][1][:, :, 1], tmp21[:],
                                            float(NP))
                issue_allreduce(1, 1)

            w2p.release()
            parkp.release()
            xtp0.release()
            ps_p2.release()
            ps_p1.release()

            # ================= pass 3: BN2 -> fused M -> head ============
            PRO3 = 6  # chunks whose b0-half runs ahead of the last AllReduce
            with (
                tc.tile_pool(name="w3p", bufs=2) as w3p,
                tc.tile_pool(name="tpark", bufs=1) as tpark,
            ):
                def load_pre2(c, b):
                    c0, tc_sz = CHUNKS[c]
                    t = w3p.tile([128, 4, tc_sz], BF16, tag=f"pre2_{b}",
                                 bufs=3, name=f"pre2_{c}_{b}")
                    nc.sync.dma_start(t[:], spill[:, b, :, c0:c0 + tc_sz])
                    return t

                def h2_act(c, b, pre2):
                    """branch-0 on ACT (1 op/slab); branch-1 on DVE (2 ops)."""
                    c0, tc_sz = CHUNKS[c]
                    h2 = w3p.tile([128, 4, tc_sz], BF16, tag=f"h2_{b}",
                                  bufs=3, name=f"h2_{c}_{b}")
                    for k in range(4):
                        if b == 0:
                            nc.scalar.activation(
                                h2[:, k, :], pre2[:, k, :], AF.Relu,
                                bias=shift_t[1][:, 0, k:k + 1],
                                scale=scale_t[1][:, 0, k:k + 1])
                        else:
                            nc.vector.tensor_scalar(
                                h2[:, k, :], pre2[:, k, :],
                                scale_t[1][:, 1, k:k + 1],
                                shift_t[1][:, 1, k:k + 1],
                                ALU.mult, ALU.add)
                            nc.vector.tensor_scalar_max(h2[:, k, :],
                                                        h2[:, k, :], 0.0)
                    return h2

                def head(c, t_sb):
                    c0, tc_sz = CHUNKS[c]
                    po = ps_o.tile([10, tc_sz], F32, tag="o", name=f"po_{c}")
                    for k in range(4):
                        nc.tensor.matmul(po[:], wh2_t[:, k, :], t_sb[:, k, :],
                                         start=(k == 0), stop=(k == 3))
                    o_sb = w3p.tile([10, tc_sz], F32, tag="o_sb", name=f"o_sb_{c}")
                    nc.scalar.activation(o_sb[:], po[:], AF.Identity,
                                         bias=bh2_sb[:, 0:1])
                    nc.sync.dma_start(outd[:, c0:c0 + tc_sz], o_sb[:])

                # prologue: b0-halves of the first PRO3 chunks run while
                # AllReduce(1,1) is still in flight; partials parked as bf16
                parked_t = {}
                for c in range(PRO3):
                    c0, tc_sz = CHUNKS[c]
                    pre2_0 = pf3_tiles.pop(c) if c in pf3_tiles else \
                        load_pre2(c, 0)
                    h2_0 = h2_act(c, 0, pre2_0)
                    tp = tpark.tile([128, 4, tc_sz], BF16, tag=f"tp_{c}",
                                    name=f"tp_{c}")
                    parked_t[c] = tp
                    for m in range(4):
                        ptl = ps_t.tile([128, tc_sz], F32, tag="t",
                                        name=f"ptl_{c}_{m}a")
                        for k in range(4):
                            nc.tensor.matmul(ptl[:], m_t[0][:, k, ts(m, 128)],
                                             h2_0[:, k, :],
                                             start=(k == 0), stop=(k == 3))
                        if m % 2 == 0:
                            nc.scalar.copy(tp[:, m, :], ptl[:])
                        else:
                            nc.vector.tensor_copy(tp[:, m, :], ptl[:])

                finish_stats(1, 1)

                # combine stage for prologue chunks (needs stats(1,1))
                for c in range(PRO3):
                    c0, tc_sz = CHUNKS[c]
                    h2_1 = h2_act(c, 1, load_pre2(c, 1))
                    t_sb = w3p.tile([128, 4, tc_sz], BF16, tag="t_sb", bufs=2,
                                    name=f"t_sb_{c}")
                    tp = parked_t.pop(c)
                    for m in range(4):
                        ptl = ps_t.tile([128, tc_sz], F32, tag="t",
                                        name=f"ptl_{c}_{m}b")
                        for k in range(4):
                            nc.tensor.matmul(ptl[:], m_t[1][:, k, ts(m, 128)],
                                             h2_1[:, k, :],
                                             start=(k == 0), stop=(k == 3))
                        # t_pre = (psum + bias) + parked, then relu
                        nc.vector.scalar_tensor_tensor(
                            t_sb[:, m, :], ptl[:], bp_sb[:, m:m + 1],
                            tp[:, m, :], ALU.add, ALU.add)
                    t_sb2 = w3p.tile([128, 4, tc_sz], BF16, tag="t_sb2", bufs=2,
                                     name=f"t_sb2_{c}")
                    nc.scalar.activation(t_sb2[:, :, :], t_sb[:, :, :], AF.Relu)
                    head(c, t_sb2)

                # steady state: full chunk in one pass
                for c in range(PRO3, C):
                    c0, tc_sz = CHUNKS[c]
                    h2_0 = h2_act(c, 0, load_pre2(c, 0))
                    h2_1 = h2_act(c, 1, load_pre2(c, 1))
                    t_sb = w3p.tile([128, 4, tc_sz], BF16, tag="t_sb", bufs=2,
                                    name=f"t_sb_{c}")
                    for m in range(4):
                        ptl = ps_t.tile([128, tc_sz], F32, tag="t",
                                        name=f"ptl_{c}_{m}")
                        for k in range(4):
                            nc.tensor.matmul(ptl[:], m_t[0][:, k, ts(m, 128)],
                                             h2_0[:, k, :],
                                             start=(k == 0), stop=False)
                        for k in range(4):
                            nc.tensor.matmul(ptl[:], m_t[1][:, k, ts(m, 128)],
                                             h2_1[:, k, :],
                                             start=False, stop=(k == 3))
                        nc.scalar.activation(t_sb[:, m, :], ptl[:], AF.Relu,
                                             bias=bp_sb[:, m:m + 1])
                    head(c, t_sb)

            ps_o.release()
            ps_t.release()

    nc.compile()
    return nc


def _get_program():
    if "nc" not in _CACHE:
        _CACHE["nc"] = _build_program()
    return _CACHE["nc"]


def kernel(**inputs):
    import ml_dtypes

    nc = _get_program()
    bf16 = ml_dtypes.bfloat16

    def shard_x(x):
        x = np.ascontiguousarray(x, dtype=np.float32).reshape(NCORES, NSH, 128)
        pad = np.zeros((NCORES, NP - NSH, 128), dtype=np.float32)
        return np.concatenate([x, pad], axis=1)  # [NCORES, NP, 128]

    xs = [shard_x(inputs["x_1"]), shard_x(inputs["x_2"])]
    # feature-major + gram layouts, bf16
    xT = [np.ascontiguousarray(x.transpose(0, 2, 1)).astype(bf16) for x in xs]
    xg = [np.ascontiguousarray(x.reshape(NCORES, 128, NSUB, 128)).astype(bf16)
          for x in xs]

    def km(w):  # [512, O] -> [128, 4, O] (contraction-major for lhsT slabs)
        O = w.shape[1]
        return np.ascontiguousarray(
            w.reshape(4, 128, O).transpose(1, 0, 2))

    def vec(v):  # [512] -> [128, 4]
        return np.ascontiguousarray(v.reshape(4, 128).T)

    f64 = np.float64
    Wh1 = np.asarray(inputs["Wh1"], f64)
    rep = {}
    for b, sfx in ((0, "1"), (1, "2")):
        rep[f"W1_{b}"] = np.asarray(inputs[f"W1_{sfx}"], np.float32).astype(bf16)
        rep[f"W2_{b}"] = km(np.asarray(inputs[f"W2_{sfx}"], np.float32))
        M = np.asarray(inputs[f"Wf_{sfx}"], f64) @ Wh1[b * 512:(b + 1) * 512, :]
        rep[f"M_{b}"] = km(M.astype(np.float32)).astype(bf16)
        for l, nm in ((0, "1"), (1, "2")):
            rep[f"g{l}_{b}"] = vec(np.asarray(inputs[f"g{nm}_{sfx}"], np.float32))
            rep[f"be{l}_{b}"] = vec(np.asarray(inputs[f"be{nm}_{sfx}"], np.float32))
    bp = (np.asarray(inputs["bf_1"], f64) @ Wh1[:512, :]
          + np.asarray(inputs["bf_2"], f64) @ Wh1[512:, :]
          + np.asarray(inputs["bh1"], f64))
    rep["BP"] = vec(bp.astype(np.float32))
    rep["WH2"] = km(np.asarray(inputs["Wh2"], np.float32)).astype(bf16)
    rep["BH2"] = np.ascontiguousarray(
        np.asarray(inputs["bh2"], np.float32).reshape(10, 1))

    in_maps = []
    for c in range(NCORES):
        m = {"xT_0": xT[0][c], "xT_1": xT[1][c],
             "xg_0": xg[0][c], "xg_1": xg[1][c]}
        m.update(rep)
        in_maps.append(m)

    res = bass_utils.run_bass_kernel_spmd(nc, in_maps, core_ids=list(range(NCORES)))
    parts = [res.results[c]["OUT"][:, :NSH] for c in range(NCORES)]
    out = np.concatenate(parts, axis=1).T
    return np.ascontiguousarray(out, dtype=np.float32)
